# revision 1
# baseline (speedup 1.0000x reference)
"""Trainium2 Bass kernel for nn_MetricBiasUpdater.

Computes, for H [4,2048,1024], B_prev [4,2048,2048], W [32,1024]:
    G    = H @ W.T                                   [4,2048,32]
    dist = |G_i|^2 + |G_j|^2 - 2 G_i.G_j             [4,2048,2048]
    out  = clip(alpha*B_prev - beta*max(dist,0), -10, 10)

Sharding: 8 cores = (batch b, row-half h).  Core (b,h) computes output rows
[h*1024,(h+1)*1024) of batch b for all 2048 columns.

Default (DSPLIT) mode: each core of a pair reads only half of H[b]^T (split
along the d contraction axis, 4 MiB instead of 8), computes a partial G, and
the pair AllReduces the small [32,2048] G^T.  The core's own row-half of G
is then selected with a partition-id-driven dynamic slice, which keeps the
SPMD program identical on every core.  With KERNEL_DSPLIT=0, each core
instead reads the full H[b]^T, with columns rotated host-side so its own
rows come first (and the output rotated back).

On-core algorithm: one augmented matmul produces -beta*dist directly:
    lhsT = -beta * [G_i; |G_i|^2; 1]   (K padded 34 -> 128 with zeros)
    rhs  =         [-2*G_j; 1; |G_j|^2]
    psum[i,j] = sum_k lhsT[k,i]*rhs[k,j] = -beta*dist[i,j]
then on the vector engine:
    t = min(psum, 0) + alpha*B_prev      == alpha*B_prev - beta*max(dist,0)
    o = max(min(t, 10), -10)

All matmul operands are bf16 (PE runs fp32 at 1/4 rate); PSUM accumulation
stays fp32, and B_prev / the output stay fp32, so the only precision loss is
on the tiny -beta*dist term (abs err ~3e-5 on this data).

SBUF partition-offset rule: sub-128-partition accesses must start at a
multiple of 32, so the two augmentation rows live at partitions 32 and 64
(rows 33..63 and 65..127 stay zero and contribute nothing to the matmul).
"""

import os
import sys

# The bass runtime drives the NeuronCores through the jax "axon" PJRT
# platform.  If a caller pinned JAX_PLATFORMS to cpu (common for running
# the pure-jax reference), undo that before jax is first imported.
if "jax" not in sys.modules:
    _jp = os.environ.get("JAX_PLATFORMS")
    if _jp is not None and "axon" not in _jp and "neuron" not in _jp:
        del os.environ["JAX_PLATFORMS"]

sys.path.insert(0, "/opt/trn_rl_repo")

import numpy as np

import concourse.bass as bass
import concourse.bacc as bacc
import concourse.mybir as mybir
from concourse.tile import TileContext
from concourse.bass_utils import run_bass_kernel_spmd

F32 = mybir.dt.float32
BF16 = mybir.dt.bfloat16
AF = mybir.ActivationFunctionType
ALU = mybir.AluOpType

B, N, D, K = 4, 2048, 1024, 32
HALF = N // 2            # rows per core
CLAMP = 10.0
N_CORES = 8
P = 128                  # partitions
JT = 512                 # moving free dim per matmul
NJ = N // JT             # 4 column chunks
KC = D // P              # 8 contraction chunks for G
R1, R2 = 32, 64          # augmentation rows (must be multiples of 32)

# D-split mode: each core of a (b,0)/(b,1) pair reads only half of H[b]^T
# (split along the d contraction axis), computes a partial G, and the pair
# AllReduces the small [32, 2048] G before the dist phase.  Halves the H
# traffic (8 -> 4 MiB per core).  The core's own row-half of G is then
# selected with a partition-id-driven dynamic slice (no host-side column
# rotation in this mode).
DSPLIT = os.environ.get("KERNEL_DSPLIT", "1") != "0"
D2 = D // 2
# Engine balance: the STT pass (PSUM read) must run on DVE at 1x rate, so
# everything else moves off DVE: memsets + the output clamp go to GPSIMD
# (1-input ops run near line rate there), with CLAMP_POOL_TILES of the 8
# clamps on GPSIMD and the rest on DVE.
CLAMP_POOL_TILES = int(os.environ.get("KERNEL_CLAMP_POOL", "8"))

_nc_cache: dict = {}


def _build_nc(alpha: float, beta: float, loop_reps: int | None = None) -> "bass.Bass":
    # Bacc (not raw Bass): its finalize() runs the legalization passes that
    # split multi-sem waits (PE instructions have a single wait slot).
    nc = bacc.Bacc(None, num_devices=N_CORES)
    d_in = D2 if DSPLIT else D
    ht = nc.dram_tensor("ht", [d_in, N], F32, kind="ExternalInput")
    wt = nc.dram_tensor("wt", [d_in, K], F32, kind="ExternalInput")
    bp_in = nc.dram_tensor("bprev", [HALF, N], F32, kind="ExternalInput")
    out = nc.dram_tensor("out", [HALF, N], F32, kind="ExternalOutput")

    with TileContext(nc) as tc:
        # Pools are shared across benchmark reps so PSUM/SBUF slot reuse
        # carries proper cross-rep dependencies (separate pools would alias
        # the same PSUM banks with no ordering).
        # PSUM budget: gp 2*[32,512] + qp 2*[1,512] + dp 2*[128,1024] = 8 banks.
        with (
            tc.tile_pool(name="persist", bufs=1) as persist,
            tc.tile_pool(name="hpool", bufs=d_in // P) as hp,
            tc.tile_pool(name="gpsum", bufs=2, space="PSUM") as gp,
            tc.tile_pool(name="qpsum", bufs=2, space="PSUM") as qp,
            tc.tile_pool(name="dpsum", bufs=2, space="PSUM") as dp,
            tc.tile_pool(
                name="bpool", bufs=int(os.environ.get("KERNEL_BPOOL", "8"))
            ) as bpool,
            tc.tile_pool(
                name="opool", bufs=int(os.environ.get("KERNEL_OPOOL", "3"))
            ) as opool,
            tc.tile_pool(name="drampool", bufs=1, space="DRAM") as drampool,
        ):
            pools = dict(
                persist=persist, hp=hp, gp=gp, qp=qp, dp=dp, bpool=bpool,
                opool=opool, drampool=drampool,
            )
            for _ in range(loop_reps or 1):
                _emit_body(nc, tc, pools, ht, wt, bp_in, out, alpha, beta)
    if not nc.is_finalized():
        nc.finalize()
    return nc


def _emit_body(nc, tc, pools, ht, wt, bp_in, out, alpha: float, beta: float):
    nb = -float(beta)
    persist, hp, gp, qp, dp = (
        pools["persist"], pools["hp"], pools["gp"], pools["qp"], pools["dp"]
    )
    bpool, opool = pools["bpool"], pools["opool"]

    # W^T in [128, n_chunks, K] layout: wt_sb[p, c, k] = W^T[c*128+p, k]
    kc_n = (D2 if DSPLIT else D) // P
    wt_sb = persist.tile([P, kc_n, K], BF16, tag="wt_sb")
    nc.gpsimd.dma_start(out=wt_sb[:], in_=wt.rearrange("(c p) k -> p c k", p=P))
    ones_sb = persist.tile([K, 1], BF16, tag="ones_sb")
    nc.gpsimd.memset(ones_sb[:], 1.0)

    # Augmented operands for the dist matmul (K padded to 128).
    # Contraction pairing: rows 0..31 G-dot term, row R1 gsq_i term,
    # row R2 gsq_j term.  Memsets on GPSIMD (cheap there, keeps DVE free).
    rhs_aug = persist.tile([P, N], BF16, tag="rhs_aug")   # rows: -2G | 1 | gsq
    lhs_aug = persist.tile([P, HALF], BF16, tag="lhs_aug")  # -b*G | -b*gsq | -b
    gsq_in = persist.tile([K, N], BF16, tag="gsq_in")     # G^2
    nc.gpsimd.memset(rhs_aug[:], 0.0)
    nc.gpsimd.memset(lhs_aug[:], 0.0)
    nc.gpsimd.memset(rhs_aug[R1 : R1 + 1, :], 1.0)
    nc.gpsimd.memset(lhs_aug[R2 : R2 + 1, :], nb)

    # ---------------- G phase ----------------
    htr = ht.rearrange("(c p) j -> c p j", p=P)
    hts = []
    for kc in range(kc_n):
        t = hp.tile([P, N], BF16, tag="ht")
        # gpsimd (SWDGE) casts f32 -> bf16 in the DMA datapath.
        nc.gpsimd.dma_start(out=t[:], in_=htr[kc])
        hts.append(t)

    if DSPLIT:
        # bf16 exchange payload: G is consumed in bf16 by the dist matmul
        # anyway, so the pair-reduce runs in bf16 and halves every hop.
        gpart_sb = persist.tile([K, N], BF16, tag="gpart_sb")
        gfull_sb = persist.tile([K, N], BF16, tag="gfull_sb")
        drampool = pools["drampool"]
        gpart_d = drampool.tile([K, N], BF16, tag="gpart_d")
        gfull_d = drampool.tile([K, N], BF16, tag="gfull_d")

    for jc in range(NJ):
        js = slice(jc * JT, (jc + 1) * JT)
        pg = gp.tile([K, JT], F32, tag="pg")
        for kc in range(kc_n):
            nc.tensor.matmul(
                pg[:],
                wt_sb[:, kc, :],
                hts[kc][:, js],
                start=(kc == 0),
                stop=(kc == kc_n - 1),
            )
        if DSPLIT:
            nc.scalar.activation(gpart_sb[:, js], pg[:], AF.Copy)
        else:
            # Own rows are columns 0:HALF (host rotated them to the front).
            nc.scalar.activation(rhs_aug[0:K, js], pg[:], AF.Copy, scale=-2.0)
            if jc * JT < HALF:
                nc.scalar.activation(lhs_aug[0:K, js], pg[:], AF.Copy, scale=nb)
            nc.scalar.activation(gsq_in[:, js], pg[:], AF.Square)

    if DSPLIT:
        nc.sync.dma_start(out=gpart_d[:], in_=gpart_sb[:])
        if os.environ.get("KERNEL_FAKE_CC"):  # TimelineSim can't model collectives
            nc.sync.dma_start(out=gfull_d[:], in_=gpart_d[:])
        else:
            nc.gpsimd.collective_compute(
                "AllReduce",
                ALU.add,
                replica_groups=[[2 * i, 2 * i + 1] for i in range(N_CORES // 2)],
                ins=[gpart_d[:]],
                outs=[gfull_d[:]],
            )
        nc.sync.dma_start(out=gfull_sb[:], in_=gfull_d[:])
        # Build the augmented operands from the reduced G.  The two big
        # G-row copies run on DVE (idle during the head); ACT does the
        # Square and the small gsq rows.  This core's own row-half is
        # selected with a partition-id-driven dynamic slice.
        nc.vector.tensor_scalar_mul(rhs_aug[0:K, :], gfull_sb[:], -2.0)
        for jc in range(NJ):  # chunked so the pq chain starts earlier
            js = slice(jc * JT, (jc + 1) * JT)
            nc.scalar.activation(gsq_in[:, js], gfull_sb[:, js], AF.Square)
        roff = (nc.vector.partition_id() & 1) * HALF
        nc.vector.tensor_scalar_mul(
            lhs_aug[0:K, 0:HALF], gfull_sb[:, bass.ds(roff, HALF)], nb
        )

    gsqf_sb = persist.tile([1, N], F32, tag="gsqf_sb")
    for jc in range(NJ):
        js = slice(jc * JT, (jc + 1) * JT)
        pq = qp.tile([1, JT], F32, tag="pq")
        nc.tensor.matmul(pq[:], ones_sb[:], gsq_in[:, js], start=True, stop=True)
        nc.scalar.activation(rhs_aug[R2 : R2 + 1, js], pq[:], AF.Copy)
        if DSPLIT:
            nc.scalar.activation(gsqf_sb[:, js], pq[:], AF.Copy)
        elif jc * JT < HALF:
            nc.scalar.activation(lhs_aug[R1 : R1 + 1, js], pq[:], AF.Copy, scale=nb)
    if DSPLIT:
        nc.scalar.activation(
            lhs_aug[R1 : R1 + 1, 0:HALF],
            gsqf_sb[:, bass.ds((nc.scalar.partition_id() & 1) * HALF, HALF)],
            AF.Copy,
            scale=nb,
        )

    # ---------------- dist + EMA phase ----------------
    for it in range(HALF // P):  # 8 i-tiles of 128 rows
        isl = slice(it * P, (it + 1) * P)
        bt = bpool.tile([P, N], F32, tag="bt")
        nc.sync.dma_start(out=bt[:], in_=bp_in[isl, :])
        if alpha != 1.0:
            nc.vector.tensor_scalar_mul(bt[:], bt[:], float(alpha))
        tt = opool.tile([P, N], F32, tag="tt")
        last = it == HALF // P - 1
        for hh in range(2):  # dist psum in two [128, 1024] pieces (2 banks each)
            hs = slice(hh * (N // 2), (hh + 1) * (N // 2))
            pd = dp.tile([P, N // 2], F32, tag="pd")
            for jc2 in range(2):
                jl = slice(jc2 * JT, (jc2 + 1) * JT)
                jg = slice(hh * (N // 2) + jc2 * JT, hh * (N // 2) + (jc2 + 1) * JT)
                nc.tensor.matmul(
                    pd[:, jl], lhs_aug[:, isl], rhs_aug[:, jg], start=True, stop=True
                )
            nc.vector.scalar_tensor_tensor(
                tt[:, hs], pd[:], 0.0, bt[:, hs], ALU.min, ALU.add
            )
            if last:
                # Final i-tile: clamp+store per half to shorten the kernel
                # tail (the drain after the last B_prev byte lands).
                oth = opool.tile([P, N // 2], F32, tag="oth")
                nc.vector.tensor_scalar(
                    oth[:], tt[:, hs], CLAMP, -CLAMP, ALU.min, ALU.max
                )
                nc.sync.dma_start(out=out[isl, hs], in_=oth[:])
        if not last:
            ot = opool.tile([P, N], F32, tag="ot")
            nc.vector.tensor_scalar(ot[:], tt[:], CLAMP, -CLAMP, ALU.min, ALU.max)
            nc.sync.dma_start(out=out[isl, :], in_=ot[:])


def _get_nc(alpha: float, beta: float) -> "bass.Bass":
    key = (alpha, beta)
    if key not in _nc_cache:
        _nc_cache[key] = _build_nc(alpha, beta)
    return _nc_cache[key]


def _make_in_maps(H, B_prev, W):
    wt_host = np.ascontiguousarray(W.T)  # [1024, 32]
    in_maps = []
    for c in range(N_CORES):
        bidx, h = divmod(c, 2)
        htb = H[bidx].T  # [1024, 2048]
        bp = B_prev[bidx, h * HALF : (h + 1) * HALF, :]
        if DSPLIT:
            # natural column order; this core reads only its d-half
            htb = htb[h * D2 : (h + 1) * D2]
            wt_c = wt_host[h * D2 : (h + 1) * D2]
        else:
            wt_c = wt_host
            if h == 1:
                htb = np.concatenate([htb[:, HALF:], htb[:, :HALF]], axis=1)
                bp = np.concatenate([bp[:, HALF:], bp[:, :HALF]], axis=1)
        in_maps.append(
            {
                "ht": np.ascontiguousarray(htb),
                "wt": np.ascontiguousarray(wt_c),
                "bprev": np.ascontiguousarray(bp),
            }
        )
    return in_maps


def _assemble(results) -> np.ndarray:
    out = np.empty((B, N, N), np.float32)
    for c in range(N_CORES):
        bidx, h = divmod(c, 2)
        r = results[c]["out"]
        if not DSPLIT and h == 1:
            r = np.concatenate([r[:, HALF:], r[:, :HALF]], axis=1)
        out[bidx, h * HALF : (h + 1) * HALF, :] = r
    return out


def _run(H, B_prev, W, alpha, beta, **rbk_kwargs):
    H = np.ascontiguousarray(np.asarray(H, dtype=np.float32))
    B_prev = np.ascontiguousarray(np.asarray(B_prev, dtype=np.float32))
    W = np.ascontiguousarray(np.asarray(W, dtype=np.float32))
    nc = _get_nc(float(alpha), float(beta))
    in_maps = _make_in_maps(H, B_prev, W)
    res = run_bass_kernel_spmd(nc, in_maps, list(range(N_CORES)), **rbk_kwargs)
    return _assemble(res.results), res


def kernel(H, B_prev, W, alpha, beta) -> np.ndarray:
    out, _ = _run(H, B_prev, W, alpha, beta)
    return out



# revision 33
# speedup vs baseline: 2.3422x; 2.3422x over previous
"""Trainium2 Bass kernel for nn_MetricBiasUpdater.

Computes, for H [4,2048,1024], B_prev [4,2048,2048], W [32,1024]:
    G    = H @ W.T                                   [4,2048,32]
    dist = |G_i|^2 + |G_j|^2 - 2 G_i.G_j             [4,2048,2048]
    out  = clip(alpha*B_prev - beta*max(dist,0), -10, 10)

Sharding: 8 cores = (batch b, row-half h).  Core (b,h) computes output rows
[h*1024,(h+1)*1024) of batch b for all 2048 columns.  Each core reads the
full H[b]^T (in a reduced dtype) and computes the full G[b] locally -- no
collectives; the redundant read is cheaper than the 3-hop DRAM round-trip
latency of a pair exchange.

Precision: the harness tolerance is rel_err < 2e-2.  B_prev is read and the
output is written in bf16 (host-side cast, ~0.2% rms each), halving the two
dominant HBM streams.  H/W enter the G matmul in fp8-e4m3 with W pre-scaled
by 1024 (descaled exactly in the PSUM->SBUF copy); G only feeds the dist
term, which contributes ~0.3% of the output magnitude, so fp8's ~3% error
on G is negligible.  Measured rel err ~2e-3 overall.

On-core algorithm: one augmented matmul produces -beta*dist directly:
    lhsT = [-beta*G_i; -beta*|G_i|^2; -beta]   (K padded 34 -> 128, zeros)
    rhs  = [-2*G_j; 1; |G_j|^2]
    psum[i,j] = -beta*dist[i,j]
then per 128-row i-tile:
    r  = Relu(-psum)            (ACT; = beta*max(dist,0), table loaded once)
    tt = bt - r                 (DVE tensor_tensor, all-bf16 SBUF -> 2x)
    o  = clip(tt, -10, 10)      (DVE tensor_scalar, all-bf16 SBUF -> 4x)
so every engine stays under the 2913 ns/tile DMA cadence (512 KiB B_prev in
+ 512 KiB out per tile at 360 B/ns).

All DMAs are issued on the sync (SP) queue in priority order -- wt, ht x4,
B_prev x8, then stores -- so the head (ht -> G -> augmented operands) is
never starved by prefetch, and the DMA engines stay saturated end to end.

SBUF partition-offset rule: sub-128-partition accesses must start at a
multiple of 32, so the two augmentation rows live at partitions 32 and 64
(rows 33..63 and 65..127 stay zero and contribute nothing to the matmul).
"""

import os
import sys

# The bass runtime drives the NeuronCores through the jax "axon" PJRT
# platform.  If a caller pinned JAX_PLATFORMS to cpu (common for running
# the pure-jax reference), undo that before jax is first imported.
if "jax" not in sys.modules:
    _jp = os.environ.get("JAX_PLATFORMS")
    if _jp is not None and "axon" not in _jp and "neuron" not in _jp:
        del os.environ["JAX_PLATFORMS"]

sys.path.insert(0, "/opt/trn_rl_repo")

import numpy as np

import concourse.bass as bass
import concourse.bacc as bacc
import concourse.mybir as mybir
from concourse.tile import TileContext
from concourse.bass_utils import run_bass_kernel_spmd

F32 = mybir.dt.float32
BF16 = mybir.dt.bfloat16
F8 = mybir.dt.float8e4
AF = mybir.ActivationFunctionType
ALU = mybir.AluOpType

B, N, D, K = 4, 2048, 1024, 32
HALF = N // 2            # rows per core
CLAMP = 10.0
N_CORES = 8
P = 128                  # partitions
JT = 512                 # moving free dim per matmul
NJ = N // JT             # 4 column chunks
NKC = D // P             # 8 contraction chunks for G
R1, R2 = 32, 64          # augmentation rows (must be multiples of 32)

# H/W dtype for the G matmul.  fp8 halves the ht DMA (1 MiB vs 2 MiB bf16);
# W is pre-scaled by WSCALE host-side so its ~1e-3 entries stay in fp8's
# normal range, and the scale is divided back out in the PSUM->SBUF copy.
# fp8 also enables DoubleRow matmuls (256-deep contraction per instruction).
HT8 = os.environ.get("KERNEL_HT8", "1") != "0"
H_DT = F8 if HT8 else BF16
WSCALE = 1024.0 if HT8 else 1.0
# One half of each i-tile's min(psum,0)+bt runs as a Pool-engine STT so the
# dist phase is paced by DMA, not ACT.  Escape hatch if hardware rejects a
# PSUM read on the Pool engine.
POOL_STT = os.environ.get("KERNEL_POOL_STT", "1") != "0"

_nc_cache: dict = {}


def _build_nc(alpha: float, beta: float, loop_reps: int | None = None) -> "bass.Bass":
    # Bacc (not raw Bass): its finalize() runs the legalization passes that
    # split multi-sem waits (PE instructions have a single wait slot).
    nc = bacc.Bacc(None, num_devices=N_CORES)
    ht = nc.dram_tensor("ht", [D, N], H_DT, kind="ExternalInput")
    # wt is host-prepacked into the [partition, chunk*K] SBUF layout so the
    # DMA is one contiguous 256 B run per partition.
    wt = nc.dram_tensor("wt", [P, NKC * K], H_DT, kind="ExternalInput")
    bp_in = nc.dram_tensor("bprev", [HALF, N], BF16, kind="ExternalInput")
    out = nc.dram_tensor("out", [HALF, N], BF16, kind="ExternalOutput")

    with TileContext(nc) as tc:
        # Pools are shared across benchmark reps so PSUM/SBUF slot reuse
        # carries proper cross-rep dependencies.
        # PSUM budget: one pool of 3*[128,1024] = 6 banks.  The G phase
        # borrows two of these tiles (all four j-chunk PSUMs live at once, so
        # no recycle stalls in the head); dp=3 lets dist tile k+1's matmuls
        # start before tile k's PSUM consumers finish.
        with (
            tc.tile_pool(name="persist", bufs=1) as persist,
            tc.tile_pool(name="dpsum", bufs=3, space="PSUM") as dp,
            tc.tile_pool(name="qpsum", bufs=2, space="PSUM") as qp,
            tc.tile_pool(name="bpool", bufs=8) as bpool,
            tc.tile_pool(name="rpool", bufs=6) as rpool,
            tc.tile_pool(name="tpool", bufs=4) as tpool,
            tc.tile_pool(name="opool", bufs=8) as opool,
        ):
            pools = dict(
                persist=persist, dp=dp, qp=qp, bpool=bpool,
                rpool=rpool, tpool=tpool, opool=opool,
            )
            for _ in range(loop_reps or 1):
                _emit_body(nc, tc, pools, ht, wt, bp_in, out, alpha, beta)
    if not nc.is_finalized():
        nc.finalize()
    return nc


def _emit_body(nc, tc, pools, ht, wt, bp_in, out, alpha: float, beta: float):
    nb = -float(beta)
    persist, dp, qp = pools["persist"], pools["dp"], pools["qp"]
    bpool, rpool, tpool, opool = (
        pools["bpool"], pools["rpool"], pools["tpool"], pools["opool"]
    )

    # ---- DMA issue order: ht then all B_prev tiles on the sync queue, the
    # small wt via SWDGE on the Pool queue (so it neither delays ht's issue
    # nor its transfer; transfer requests hit the shared DMA engines in
    # roughly this order and the G phase is never starved by prefetch).
    wt_sb = persist.tile([P, NKC, K], H_DT, tag="wt_sb")
    nc.gpsimd.dma_start(out=wt_sb[:], in_=wt.rearrange("p (c k) -> p c k", c=NKC))

    ht_sb = persist.tile([P, NKC, N], H_DT, tag="ht_sb")
    htr = ht.rearrange("(c p) j -> p c j", p=P)
    for g in range(4):
        nc.sync.dma_start(
            out=ht_sb[:, 2 * g : 2 * g + 2, :], in_=htr[:, 2 * g : 2 * g + 2, :]
        )

    bts = []
    for it in range(HALF // P):
        btile = bpool.tile([P, N], BF16, tag="bt")
        nc.sync.dma_start(out=btile[:], in_=bp_in[it * P : (it + 1) * P, :])
        bts.append(btile)

    # ---- one-time constants / zero padding (Pool engine; off the DMA path)
    ones_sb = persist.tile([K, 1], BF16, tag="ones_sb")
    nc.gpsimd.memset(ones_sb[:], 1.0)
    # rhs_aug rows: 0..31 = -2*G_j | 32 = 1 | 64 = |G_j|^2 ; rest zero.
    # lhs_aug rows: 0..31 = -b*G_i | 32 = -b*|G_i|^2 | 64 = -b ; rest zero.
    rhs_aug = persist.tile([P, N], BF16, tag="rhs_aug")
    lhs_aug = persist.tile([P, HALF], BF16, tag="lhs_aug")
    gsq_in = persist.tile([K, N], BF16, tag="gsq_in")
    nc.gpsimd.memset(rhs_aug[:], 0.0)
    nc.gpsimd.memset(lhs_aug[:], 0.0)
    nc.gpsimd.memset(rhs_aug[R1 : R1 + 1, :], 1.0)
    nc.gpsimd.memset(lhs_aug[R2 : R2 + 1, :], nb)

    # Warm the ACT function table (Copy+Relu) at t~0 on a 1-element dummy so
    # the 1283 ns LoadActFuncSet overlaps the ht DMA instead of stalling the
    # first G-phase copy.
    warm = persist.tile([1, 1], BF16, tag="warm")
    nc.gpsimd.memset(warm[:], 0.0)
    nc.scalar.activation(warm[:], warm[:], AF.Relu)

    # ---------------- G phase (full G[b], computed locally) ----------------
    # psum = WSCALE * G^T[k, js]; the PSUM->SBUF copy descales and applies
    # the -2.  fp8 DoubleRow contracts a kc-pair (256 rows) per matmul.
    # PSUM->SBUF copies alternate ACT/DVE per chunk so neither engine
    # serializes the G tail.
    # Two borrowed dist-PSUM tiles hold all four j-chunk G PSUMs at once
    # (G in partitions 0..31, the gsq row-sum in partition 32 of the same
    # banks), so the G tail has no PSUM-recycle stalls.
    pga = dp.tile([P, N // 2], F32, tag="pd")
    pgb = dp.tile([P, N // 2], F32, tag="pd")
    pgs = [pga, pgb]
    # All 16 G matmuls are emitted before any PSUM consumer: PE runs in
    # program order, so interleaving the (ACT/DVE-gated) gsq ones-matmuls
    # here would head-of-line-block the later j-chunks' G matmuls.
    for jc in range(NJ):
        js = slice(jc * JT, (jc + 1) * JT)
        pg = pgs[jc // 2][:, (jc % 2) * JT : (jc % 2 + 1) * JT]
        if HT8:
            for g in range(4):
                nc.tensor.matmul(
                    pg[0:K, :],
                    wt_sb[:, 2 * g : 2 * g + 2, :],
                    ht_sb[:, 2 * g : 2 * g + 2, js],
                    start=(g == 0),
                    stop=(g == 3),
                    perf_mode=mybir.MatmulPerfMode.DoubleRow,
                )
        else:
            for kc in range(NKC):
                nc.tensor.matmul(
                    pg[0:K, :],
                    wt_sb[:, kc, :],
                    ht_sb[:, kc, js],
                    start=(kc == 0),
                    stop=(kc == NKC - 1),
                )
    # Per-chunk consumer chains, engine-balanced so the last chunk's chain
    # (psum -> copy -> square -> ones-mm -> R2 -> lhs) stays on fast engines
    # and no single engine serializes the head:
    #   copy+square: even chunks on ACT, odd chunks on DVE
    # All copies/squares are emitted before any pq ones-matmul: two of the
    # pq PSUMs live in spare partitions of pga, and emitting a pq write
    # before a later chunk's copy read would add a false tile-level RAW dep.
    for jc in range(NJ):
        js = slice(jc * JT, (jc + 1) * JT)
        pg = pgs[jc // 2][:, (jc % 2) * JT : (jc % 2 + 1) * JT]
        # Pool cannot touch PSUM on real hardware, so the PSUM->SBUF copies
        # live on ACT (even chunks) / DVE (odd chunks); Pool squares the
        # early chunks from SBUF (4 G^2, descaled in the R2 copy), and the
        # late chunks square straight from PSUM on ACT/DVE.
        if jc % 2 == 0:
            nc.scalar.activation(
                rhs_aug[0:K, js], pg[0:K, :], AF.Copy, scale=-2.0 / WSCALE
            )
        else:
            nc.vector.tensor_scalar_mul(rhs_aug[0:K, js], pg[0:K, :], -2.0 / WSCALE)
        if jc == 0:
            nc.gpsimd.tensor_tensor(
                gsq_in[:, js], rhs_aug[0:K, js], rhs_aug[0:K, js], ALU.mult
            )
        elif jc == 3:
            nc.vector.tensor_tensor(
                gsq_in[:, js], pg[0:K, :], pg[0:K, :], ALU.mult
            )
        else:
            nc.scalar.activation(
                gsq_in[:, js], pg[0:K, :], AF.Square, scale=1.0 / WSCALE
            )
    # gsq row sums: pq j0/j1 in the qp pool, j2/j3 in pga's partition 32
    # (same banks as the retired j0/j1 G PSUMs).  R2 descale: j0 on Pool,
    # j1/j3 on DVE, j2 on ACT.
    pq0 = qp.tile([1, JT], F32, tag="pq")
    pq1 = qp.tile([1, JT], F32, tag="pq")
    pqs = [pq0[:], pq1[:], pga[K : K + 1, 0:JT], pga[K : K + 1, JT : 2 * JT]]
    for jc in range(NJ):
        js = slice(jc * JT, (jc + 1) * JT)
        nc.tensor.matmul(pqs[jc], ones_sb[:], gsq_in[:, js], start=True, stop=True)
        r2_scale = [0.25, 1.0, 1.0, 1.0 / (WSCALE * WSCALE)][jc]
        if jc == 1:
            nc.vector.tensor_scalar_mul(rhs_aug[R2 : R2 + 1, js], pqs[jc], r2_scale)
        else:
            nc.scalar.activation(
                rhs_aug[R2 : R2 + 1, js], pqs[jc], AF.Copy, scale=r2_scale
            )

    # lhs_aug: this core's own row-half, selected with a partition-id-driven
    # dynamic slice (keeps the SPMD program identical on every core).
    roff = (nc.vector.partition_id() & 1) * HALF
    nc.vector.tensor_scalar_mul(
        lhs_aug[0:K, 0:HALF], rhs_aug[0:K, bass.ds(roff, HALF)], float(beta) / 2.0
    )
    nc.vector.tensor_scalar_mul(
        lhs_aug[R1 : R1 + 1, 0:HALF], rhs_aug[R2 : R2 + 1, bass.ds(roff, HALF)], nb
    )

    # ---------------- dist + EMA phase ----------------
    for it in range(HALF // P):  # 8 i-tiles of 128 rows
        isl = slice(it * P, (it + 1) * P)
        bt = bts[it]
        if alpha != 1.0:
            nc.vector.tensor_scalar_mul(bt[:], bt[:], float(alpha))
        tt = tpool.tile([P, N], BF16, tag="tt")
        for hh in range(2):  # dist psum in two [128, 1024] pieces (2 banks each)
            hs = slice(hh * (N // 2), (hh + 1) * (N // 2))
            pd = dp.tile([P, N // 2], F32, tag="pd")
            for jc2 in range(2):
                jl = slice(jc2 * JT, (jc2 + 1) * JT)
                jg = slice(hh * (N // 2) + jc2 * JT, hh * (N // 2) + (jc2 + 1) * JT)
                nc.tensor.matmul(
                    pd[:, jl], lhs_aug[:, isl], rhs_aug[:, jg], start=True, stop=True
                )
            # tt = bt + min(psum, 0) = bt - beta*max(dist, 0).  Pool cannot
            # read PSUM on hardware, so the PSUM consumption is split ACT:DVE
            # at 3:1 -- ACT Relu(-psum) on half 0 and the first 512 columns
            # of half 1 (DVE subtracts bt), DVE STT straight from PSUM on the
            # remaining 512 -- and the clamps split Pool/DVE.  Per tile:
            # ACT 1650, DVE 1873, Pool 1612, under the ~2.1us chain budget.
            if hh == 0:
                r16 = rpool.tile([P, N // 2], BF16, tag="r16")
                nc.scalar.activation(r16[:], pd[:], AF.Relu, scale=-1.0)
                nc.vector.tensor_tensor(tt[:, hs], bt[:, hs], r16[:], ALU.subtract)
            else:
                r16 = rpool.tile([P, N // 2], BF16, tag="r16")
                nc.scalar.activation(r16[:, 0:JT], pd[:, 0:JT], AF.Relu, scale=-1.0)
                nc.vector.tensor_tensor(
                    tt[:, N // 2 : N // 2 + JT], bt[:, N // 2 : N // 2 + JT],
                    r16[:, 0:JT], ALU.subtract,
                )
                nc.vector.scalar_tensor_tensor(
                    tt[:, N // 2 + JT : N], pd[:, JT : 2 * JT], 0.0,
                    bt[:, N // 2 + JT : N], ALU.min, ALU.add,
                )
            oth = opool.tile([P, N // 2], BF16, tag="oth")
            if hh == 0:
                nc.gpsimd.tensor_scalar(
                    oth[:], tt[:, hs], CLAMP, -CLAMP, ALU.min, ALU.max
                )
            else:
                nc.vector.tensor_scalar(
                    oth[:], tt[:, hs], CLAMP, -CLAMP, ALU.min, ALU.max
                )
            nc.sync.dma_start(out=out[isl, hs], in_=oth[:])


def _get_nc(alpha: float, beta: float) -> "bass.Bass":
    key = (alpha, beta)
    if key not in _nc_cache:
        _nc_cache[key] = _build_nc(alpha, beta)
    return _nc_cache[key]


def _make_in_maps(H, B_prev, W):
    h_np = mybir.dt.np(H_DT)
    # wt prepacked to the SBUF layout: wt_host[p, c*K+k] = WSCALE*W^T[c*P+p, k]
    wtt = (W.T * WSCALE).reshape(NKC, P, K)  # [c, p, k]
    wt_host = np.ascontiguousarray(wtt.transpose(1, 0, 2).reshape(P, NKC * K)).astype(
        h_np
    )
    bf_np = mybir.dt.np(BF16)
    in_maps = []
    for c in range(N_CORES):
        bidx, h = divmod(c, 2)
        htb = np.ascontiguousarray(H[bidx].T).astype(h_np)  # [1024, 2048]
        bp = B_prev[bidx, h * HALF : (h + 1) * HALF, :].astype(bf_np)
        in_maps.append(
            {
                "ht": htb,
                "wt": wt_host,
                "bprev": np.ascontiguousarray(bp),
            }
        )
    return in_maps


def _assemble(results) -> np.ndarray:
    out = np.empty((B, N, N), np.float32)
    for c in range(N_CORES):
        bidx, h = divmod(c, 2)
        out[bidx, h * HALF : (h + 1) * HALF, :] = results[c]["out"].astype(np.float32)
    return out


def _run(H, B_prev, W, alpha, beta, **rbk_kwargs):
    H = np.ascontiguousarray(np.asarray(H, dtype=np.float32))
    B_prev = np.ascontiguousarray(np.asarray(B_prev, dtype=np.float32))
    W = np.ascontiguousarray(np.asarray(W, dtype=np.float32))
    nc = _get_nc(float(alpha), float(beta))
    in_maps = _make_in_maps(H, B_prev, W)
    res = run_bass_kernel_spmd(nc, in_maps, list(range(N_CORES)), **rbk_kwargs)
    return _assemble(res.results), res


def kernel(H, B_prev, W, alpha, beta) -> np.ndarray:
    out, _ = _run(H, B_prev, W, alpha, beta)
    return out


# revision 45
# speedup vs baseline: 2.3679x; 1.0110x over previous
"""Trainium2 Bass kernel for nn_MetricBiasUpdater.

Computes, for H [4,2048,1024], B_prev [4,2048,2048], W [32,1024]:
    G    = H @ W.T                                   [4,2048,32]
    dist = |G_i|^2 + |G_j|^2 - 2 G_i.G_j             [4,2048,2048]
    out  = clip(alpha*B_prev - beta*max(dist,0), -10, 10)

Sharding: 8 cores = (batch b, row-half h).  Core (b,h) computes output rows
[h*1024,(h+1)*1024) of batch b for all 2048 columns.  Each core reads the
full H[b]^T (in a reduced dtype) and computes the full G[b] locally -- no
collectives; the redundant read is cheaper than the 3-hop DRAM round-trip
latency of a pair exchange.

Precision: the harness tolerance is rel_err < 2e-2.  B_prev is read and the
output is written in bf16 (host-side cast, ~0.2% rms each), halving the two
dominant HBM streams.  H/W enter the G matmul in fp8-e4m3 with W pre-scaled
by 1024 (descaled exactly in the PSUM->SBUF copy); G only feeds the dist
term, which contributes ~0.3% of the output magnitude, so fp8's ~3% error
on G is negligible.  Measured rel err ~2e-3 overall.

On-core algorithm: one augmented matmul produces -beta*dist directly:
    lhsT = [-beta*G_i; -beta*|G_i|^2; -beta]   (K padded 34 -> 128, zeros)
    rhs  = [-2*G_j; 1; |G_j|^2]
    psum[i,j] = -beta*dist[i,j]
then per 128-row i-tile the PSUM is turned into clip(bt - beta*max(dist,0))
with the work spread over ACT (Relu from PSUM), DVE (bf16 subtract, STT,
clamp) and Pool (bf16 clamp), so the pipeline stays close to the DMA drain
rate (1 MiB/tile at 360 B/ns).

All load/store DMAs are issued on the sync (SP) queue in priority order --
ht x4, B_prev x8, then stores -- (wt goes via SWDGE on the Pool queue) so
the head (ht -> G -> augmented operands) is never starved by prefetch and
the DMA engines stay saturated end to end.

SBUF partition-offset rule: sub-128-partition accesses must start at a
multiple of 32, so the two augmentation rows live at partitions 32 and 64
(rows 33..63 and 65..127 stay zero and contribute nothing to the matmul).
"""

import os
import sys

# The bass runtime drives the NeuronCores through the jax "axon" PJRT
# platform.  If a caller pinned JAX_PLATFORMS to cpu (common for running
# the pure-jax reference), undo that before jax is first imported.
if "jax" not in sys.modules:
    _jp = os.environ.get("JAX_PLATFORMS")
    if _jp is not None and "axon" not in _jp and "neuron" not in _jp:
        del os.environ["JAX_PLATFORMS"]

sys.path.insert(0, "/opt/trn_rl_repo")

import numpy as np

import concourse.bass as bass
import concourse.bacc as bacc
import concourse.mybir as mybir
from concourse.tile import TileContext
from concourse.bass_utils import run_bass_kernel_spmd

F32 = mybir.dt.float32
BF16 = mybir.dt.bfloat16
F8 = mybir.dt.float8e4
AF = mybir.ActivationFunctionType
ALU = mybir.AluOpType

B, N, D, K = 4, 2048, 1024, 32
HALF = N // 2            # rows per core
CLAMP = 10.0
N_CORES = 8
P = 128                  # partitions
JT = 512                 # moving free dim per matmul
NJ = N // JT             # 4 column chunks
NKC = D // P             # 8 contraction chunks for G
R1, R2 = 32, 64          # augmentation rows (must be multiples of 32)

# H/W dtype for the G matmul.  fp8 halves the ht DMA (1 MiB vs 2 MiB bf16);
# W is pre-scaled by WSCALE host-side so its ~1e-3 entries stay in fp8's
# normal range, and the scale is divided back out in the PSUM->SBUF copy.
# fp8 also enables DoubleRow matmuls (256-deep contraction per instruction).
HT8 = os.environ.get("KERNEL_HT8", "1") != "0"
H_DT = F8 if HT8 else BF16
WSCALE = 1024.0 if HT8 else 1.0
_nc_cache: dict = {}


def _build_nc(alpha: float, beta: float, loop_reps: int | None = None) -> "bass.Bass":
    # Bacc (not raw Bass): its finalize() runs the legalization passes that
    # split multi-sem waits (PE instructions have a single wait slot).
    nc = bacc.Bacc(None, num_devices=N_CORES)
    ht = nc.dram_tensor("ht", [D, N], H_DT, kind="ExternalInput")
    # wt is host-prepacked into the [partition, chunk*K] SBUF layout so the
    # DMA is one contiguous 256 B run per partition.
    wt = nc.dram_tensor("wt", [P, NKC * K], H_DT, kind="ExternalInput")
    bp_in = nc.dram_tensor("bprev", [HALF, N], BF16, kind="ExternalInput")
    out = nc.dram_tensor("out", [HALF, N], BF16, kind="ExternalOutput")

    with TileContext(nc) as tc:
        # Pools are shared across benchmark reps so PSUM/SBUF slot reuse
        # carries proper cross-rep dependencies.
        # PSUM budget: one pool of 3*[128,1024] = 6 banks.  The G phase
        # borrows two of these tiles (all four j-chunk PSUMs live at once, so
        # no recycle stalls in the head); dp=3 lets dist tile k+1's matmuls
        # start before tile k's PSUM consumers finish.
        with (
            tc.tile_pool(name="persist", bufs=1) as persist,
            tc.tile_pool(name="dpsum", bufs=3, space="PSUM") as dp,
            tc.tile_pool(name="qpsum", bufs=2, space="PSUM") as qp,
            tc.tile_pool(name="bpool", bufs=8) as bpool,
            tc.tile_pool(name="rpool", bufs=6) as rpool,
            tc.tile_pool(name="tpool", bufs=4) as tpool,
            tc.tile_pool(name="opool", bufs=8) as opool,
        ):
            pools = dict(
                persist=persist, dp=dp, qp=qp, bpool=bpool,
                rpool=rpool, tpool=tpool, opool=opool,
            )
            for _ in range(loop_reps or 1):
                _emit_body(nc, tc, pools, ht, wt, bp_in, out, alpha, beta)
    if not nc.is_finalized():
        nc.finalize()
    return nc


def _emit_body(nc, tc, pools, ht, wt, bp_in, out, alpha: float, beta: float):
    nb = -float(beta)
    persist, dp, qp = pools["persist"], pools["dp"], pools["qp"]
    bpool, rpool, tpool, opool = (
        pools["bpool"], pools["rpool"], pools["tpool"], pools["opool"]
    )

    # ---- DMA issue order: ht then all B_prev tiles on the sync queue, the
    # small wt via SWDGE on the Pool queue (so it neither delays ht's issue
    # nor its transfer; transfer requests hit the shared DMA engines in
    # roughly this order and the G phase is never starved by prefetch).
    wt_sb = persist.tile([P, NKC, K], H_DT, tag="wt_sb")
    nc.gpsimd.dma_start(out=wt_sb[:], in_=wt.rearrange("p (c k) -> p c k", c=NKC))

    ht_sb = persist.tile([P, NKC, N], H_DT, tag="ht_sb")
    htr = ht.rearrange("(c p) j -> p c j", p=P)
    for g in range(4):
        nc.sync.dma_start(
            out=ht_sb[:, 2 * g : 2 * g + 2, :], in_=htr[:, 2 * g : 2 * g + 2, :]
        )

    bts = []
    for it in range(HALF // P):
        btile = bpool.tile([P, N], BF16, tag="bt")
        nc.sync.dma_start(out=btile[:], in_=bp_in[it * P : (it + 1) * P, :])
        bts.append(btile)

    # ---- one-time constants / zero padding (Pool engine; off the DMA path)
    ones_sb = persist.tile([K, 1], BF16, tag="ones_sb")
    nc.gpsimd.memset(ones_sb[:], 1.0)
    # rhs_aug rows: 0..31 = -2*G_j | 32 = 1 | 64 = |G_j|^2 ; rest zero.
    # lhs_aug rows: 0..31 = -b*G_i | 32 = -b*|G_i|^2 | 64 = -b ; rest zero.
    rhs_aug = persist.tile([P, N], BF16, tag="rhs_aug")
    lhs_aug = persist.tile([P, HALF], BF16, tag="lhs_aug")
    gsq_in = persist.tile([K, N], BF16, tag="gsq_in")
    nc.gpsimd.memset(rhs_aug[:], 0.0)
    nc.gpsimd.memset(lhs_aug[:], 0.0)
    nc.gpsimd.memset(rhs_aug[R1 : R1 + 1, :], 1.0)
    nc.gpsimd.memset(lhs_aug[R2 : R2 + 1, :], nb)

    # Warm the ACT function table (Copy+Relu) at t~0 on a 1-element dummy so
    # the 1283 ns LoadActFuncSet overlaps the ht DMA instead of stalling the
    # first G-phase copy.
    warm = persist.tile([1, 1], BF16, tag="warm")
    nc.gpsimd.memset(warm[:], 0.0)
    nc.scalar.activation(warm[:], warm[:], AF.Relu)

    # ---------------- G phase (full G[b], computed locally) ----------------
    # psum = WSCALE * G^T[k, js]; the PSUM->SBUF copy descales and applies
    # the -2.  fp8 DoubleRow contracts a kc-pair (256 rows) per matmul.
    # PSUM->SBUF copies alternate ACT/DVE per chunk so neither engine
    # serializes the G tail.
    # Two borrowed dist-PSUM tiles hold all four j-chunk G PSUMs at once
    # (G in partitions 0..31, the gsq row-sum in partition 32 of the same
    # banks), so the G tail has no PSUM-recycle stalls.
    pga = dp.tile([P, N // 2], F32, tag="pd")
    pgb = dp.tile([P, N // 2], F32, tag="pd")
    pgs = [pga, pgb]
    # All 16 G matmuls are emitted before any PSUM consumer: PE runs in
    # program order, so interleaving the (ACT/DVE-gated) gsq ones-matmuls
    # here would head-of-line-block the later j-chunks' G matmuls.
    for jc in range(NJ):
        js = slice(jc * JT, (jc + 1) * JT)
        pg = pgs[jc // 2][:, (jc % 2) * JT : (jc % 2 + 1) * JT]
        if HT8:
            for g in range(4):
                nc.tensor.matmul(
                    pg[0:K, :],
                    wt_sb[:, 2 * g : 2 * g + 2, :],
                    ht_sb[:, 2 * g : 2 * g + 2, js],
                    start=(g == 0),
                    stop=(g == 3),
                    perf_mode=mybir.MatmulPerfMode.DoubleRow,
                )
        else:
            for kc in range(NKC):
                nc.tensor.matmul(
                    pg[0:K, :],
                    wt_sb[:, kc, :],
                    ht_sb[:, kc, js],
                    start=(kc == 0),
                    stop=(kc == NKC - 1),
                )
    # Per-chunk consumer chains, hand-ordered to steer the list scheduler:
    # the j3 chain (psum -> square -> ones-mm -> R2) is the lhs barrier's
    # critical path, so its square reads PSUM directly (skipping the copy
    # hop) and is emitted before copy-j3; its ones-mm and R2 are emitted
    # before the other chunks'.  copies: ACT (even) / DVE (odd); squares:
    # j0 Pool from SBUF, j1/j2 ACT from PSUM, j3 DVE from PSUM; R2 descale:
    # j0/j2 ACT, j1/j3 DVE.  gsq scale varies per source (see r2_scale).
    def _copy(jc):
        js = slice(jc * JT, (jc + 1) * JT)
        pg = pgs[jc // 2][:, (jc % 2) * JT : (jc % 2 + 1) * JT]
        if jc % 2 == 0:
            nc.scalar.activation(
                rhs_aug[0:K, js], pg[0:K, :], AF.Copy, scale=-2.0 / WSCALE
            )
        else:
            nc.vector.tensor_scalar_mul(rhs_aug[0:K, js], pg[0:K, :], -2.0 / WSCALE)

    def _square(jc):
        js = slice(jc * JT, (jc + 1) * JT)
        pg = pgs[jc // 2][:, (jc % 2) * JT : (jc % 2 + 1) * JT]
        if jc == 0:
            nc.gpsimd.tensor_tensor(
                gsq_in[:, js], rhs_aug[0:K, js], rhs_aug[0:K, js], ALU.mult
            )
        elif jc == 3:
            # TensorTensor may read only one PSUM operand on hardware, so
            # j3 squares the SBUF copy (-2G)^2 = 4 G^2 instead.
            nc.vector.tensor_tensor(
                gsq_in[:, js], rhs_aug[0:K, js], rhs_aug[0:K, js], ALU.mult
            )
        else:
            nc.scalar.activation(
                gsq_in[:, js], pg[0:K, :], AF.Square, scale=1.0 / WSCALE
            )

    pq0 = qp.tile([1, JT], F32, tag="pq")
    pq1 = qp.tile([1, JT], F32, tag="pq")
    pqs = [pq0[:], pq1[:], pga[K : K + 1, 0:JT], pga[K : K + 1, JT : 2 * JT]]

    def _r2(jc):
        js = slice(jc * JT, (jc + 1) * JT)
        nc.tensor.matmul(pqs[jc], ones_sb[:], gsq_in[:, js], start=True, stop=True)
        r2_scale = [0.25, 1.0, 1.0, 0.25][jc]
        if jc == 1:
            nc.vector.tensor_scalar_mul(rhs_aug[R2 : R2 + 1, js], pqs[jc], r2_scale)
        else:
            nc.scalar.activation(
                rhs_aug[R2 : R2 + 1, js], pqs[jc], AF.Copy, scale=r2_scale
            )

    for jc in range(NJ):
        _copy(jc)
        _square(jc)
    for jc in range(NJ):
        _r2(jc)

    # lhs_aug: this core's own row-half, selected with a partition-id-driven
    # dynamic slice (keeps the SPMD program identical on every core).
    roff = (nc.vector.partition_id() & 1) * HALF
    nc.vector.tensor_scalar_mul(
        lhs_aug[0:K, 0:HALF], rhs_aug[0:K, bass.ds(roff, HALF)], float(beta) / 2.0
    )
    nc.vector.tensor_scalar_mul(
        lhs_aug[R1 : R1 + 1, 0:HALF], rhs_aug[R2 : R2 + 1, bass.ds(roff, HALF)], nb
    )

    # ---------------- dist + EMA phase ----------------
    for it in range(HALF // P):  # 8 i-tiles of 128 rows
        isl = slice(it * P, (it + 1) * P)
        bt = bts[it]
        if alpha != 1.0:
            nc.vector.tensor_scalar_mul(bt[:], bt[:], float(alpha))
        tt = tpool.tile([P, N], BF16, tag="tt")
        for hh in range(2):  # dist psum in two [128, 1024] pieces (2 banks each)
            hs = slice(hh * (N // 2), (hh + 1) * (N // 2))
            pd = dp.tile([P, N // 2], F32, tag="pd")
            for jc2 in range(2):
                jl = slice(jc2 * JT, (jc2 + 1) * JT)
                jg = slice(hh * (N // 2) + jc2 * JT, hh * (N // 2) + (jc2 + 1) * JT)
                nc.tensor.matmul(
                    pd[:, jl], lhs_aug[:, isl], rhs_aug[:, jg], start=True, stop=True
                )
            # tt = bt + min(psum, 0) = bt - beta*max(dist, 0).  Pool cannot
            # read PSUM on hardware, so the 2048 PSUM columns are consumed
            # ACT:DVE at roughly 5:1 -- ACT Relu(-psum) on half 0 plus 704
            # columns of half 1 (DVE subtracts bt from each), a DVE STT
            # straight from PSUM on the remaining 320 -- and the clamps
            # split Pool (half 0) / DVE (half 1), balancing all three
            # engines under the ~1.9us/tile pipeline cadence.
            if hh == 0:
                r16 = rpool.tile([P, N // 2], BF16, tag="r16")
                nc.scalar.activation(r16[:], pd[:], AF.Relu, scale=-1.0)
                nc.vector.tensor_tensor(tt[:, hs], bt[:, hs], r16[:], ALU.subtract)
            else:
                RW = 704  # ACT-relu'd columns of half 1; DVE STTs the rest
                r16 = rpool.tile([P, N // 2], BF16, tag="r16")
                nc.scalar.activation(r16[:, 0:RW], pd[:, 0:RW], AF.Relu, scale=-1.0)
                nc.vector.tensor_tensor(
                    tt[:, N // 2 : N // 2 + RW], bt[:, N // 2 : N // 2 + RW],
                    r16[:, 0:RW], ALU.subtract,
                )
                nc.vector.scalar_tensor_tensor(
                    tt[:, N // 2 + RW : N], pd[:, RW : 2 * JT], 0.0,
                    bt[:, N // 2 + RW : N], ALU.min, ALU.add,
                )
            oth = opool.tile([P, N // 2], BF16, tag="oth")
            if hh == 0:
                nc.gpsimd.tensor_scalar(
                    oth[:], tt[:, hs], CLAMP, -CLAMP, ALU.min, ALU.max
                )
            else:
                nc.vector.tensor_scalar(
                    oth[:], tt[:, hs], CLAMP, -CLAMP, ALU.min, ALU.max
                )
            nc.sync.dma_start(out=out[isl, hs], in_=oth[:])


def _get_nc(alpha: float, beta: float) -> "bass.Bass":
    key = (alpha, beta)
    if key not in _nc_cache:
        _nc_cache[key] = _build_nc(alpha, beta)
    return _nc_cache[key]


def _make_in_maps(H, B_prev, W):
    h_np = mybir.dt.np(H_DT)
    # wt prepacked to the SBUF layout: wt_host[p, c*K+k] = WSCALE*W^T[c*P+p, k]
    wtt = (W.T * WSCALE).reshape(NKC, P, K)  # [c, p, k]
    wt_host = np.ascontiguousarray(wtt.transpose(1, 0, 2).reshape(P, NKC * K)).astype(
        h_np
    )
    bf_np = mybir.dt.np(BF16)
    in_maps = []
    for c in range(N_CORES):
        bidx, h = divmod(c, 2)
        htb = np.ascontiguousarray(H[bidx].T).astype(h_np)  # [1024, 2048]
        bp = B_prev[bidx, h * HALF : (h + 1) * HALF, :].astype(bf_np)
        in_maps.append(
            {
                "ht": htb,
                "wt": wt_host,
                "bprev": np.ascontiguousarray(bp),
            }
        )
    return in_maps


def _assemble(results) -> np.ndarray:
    out = np.empty((B, N, N), np.float32)
    for c in range(N_CORES):
        bidx, h = divmod(c, 2)
        out[bidx, h * HALF : (h + 1) * HALF, :] = results[c]["out"].astype(np.float32)
    return out


def _run(H, B_prev, W, alpha, beta, **rbk_kwargs):
    H = np.ascontiguousarray(np.asarray(H, dtype=np.float32))
    B_prev = np.ascontiguousarray(np.asarray(B_prev, dtype=np.float32))
    W = np.ascontiguousarray(np.asarray(W, dtype=np.float32))
    nc = _get_nc(float(alpha), float(beta))
    in_maps = _make_in_maps(H, B_prev, W)
    res = run_bass_kernel_spmd(nc, in_maps, list(range(N_CORES)), **rbk_kwargs)
    return _assemble(res.results), res


def kernel(H, B_prev, W, alpha, beta) -> np.ndarray:
    out, _ = _run(H, B_prev, W, alpha, beta)
    return out


# revision 61
# speedup vs baseline: 2.4248x; 1.0240x over previous
"""Trainium2 Bass kernel for nn_MetricBiasUpdater.

Computes, for H [4,2048,1024], B_prev [4,2048,2048], W [32,1024]:
    G    = H @ W.T                                   [4,2048,32]
    dist = |G_i|^2 + |G_j|^2 - 2 G_i.G_j             [4,2048,2048]
    out  = clip(alpha*B_prev - beta*max(dist,0), -10, 10)

Sharding: 8 cores = (batch b, row-half h).  Core (b,h) computes output rows
[h*1024,(h+1)*1024) of batch b for all 2048 columns.  Each core reads the
full H[b]^T (in a reduced dtype) and computes the full G[b] locally -- no
collectives; the redundant read is cheaper than the 3-hop DRAM round-trip
latency of a pair exchange.  The host rotates each core's columns so its
own row-half is always columns [0:1024): the lhs operand build then depends
statically on only the first two G column chunks, and the first dist-phase
matmuls overlap the remaining chunks' gsq work.

Precision: the harness tolerance is rel_err < 2e-2.  B_prev is read and the
output is written in bf16 (host-side cast, ~0.2% rms each), halving the two
dominant HBM streams.  H/W enter the G matmul in fp8-e4m3 with W pre-scaled
by 1024 (descaled exactly in the PSUM->SBUF copy); G only feeds the dist
term, which contributes ~0.3% of the output magnitude, so fp8's ~3% error
on G is negligible.  Measured rel err ~3.5e-3 overall.

On-core algorithm: one augmented matmul produces -beta*dist directly:
    lhsT = [-beta*G_i; -beta*|G_i|^2; -beta]   (K padded 34 -> 128, zeros)
    rhs  = [-2*G_j; 1; |G_j|^2]
    psum[i,j] = -beta*dist[i,j]
then per 128-row i-tile the PSUM is turned into clip(bt - beta*max(dist,0))
with the work spread over ACT (Relu from PSUM), DVE (bf16 subtract, STT,
clamp) and Pool (bf16 clamp), so the pipeline stays close to the DMA drain
rate (1 MiB/tile at 360 B/ns).

All load/store DMAs are issued on the sync (SP) queue in priority order --
ht x4, B_prev x8, then stores -- (wt goes via SWDGE on the Pool queue) so
the head (ht -> G -> augmented operands) is never starved by prefetch and
the DMA engines stay saturated end to end.

SBUF partition-offset rule: sub-128-partition accesses must start at a
multiple of 32, so the two augmentation rows live at partitions 32 and 64
(rows 33..63 and 65..127 stay zero and contribute nothing to the matmul).
"""

import os
import sys

# The bass runtime drives the NeuronCores through the jax "axon" PJRT
# platform.  If a caller pinned JAX_PLATFORMS to cpu (common for running
# the pure-jax reference), undo that before jax is first imported.
if "jax" not in sys.modules:
    _jp = os.environ.get("JAX_PLATFORMS")
    if _jp is not None and "axon" not in _jp and "neuron" not in _jp:
        del os.environ["JAX_PLATFORMS"]

sys.path.insert(0, "/opt/trn_rl_repo")

import numpy as np

import concourse.bass as bass
import concourse.bacc as bacc
import concourse.mybir as mybir
from concourse.tile import TileContext
from concourse.bass_utils import run_bass_kernel_spmd

F32 = mybir.dt.float32
BF16 = mybir.dt.bfloat16
F8 = mybir.dt.float8e4
AF = mybir.ActivationFunctionType
ALU = mybir.AluOpType

B, N, D, K = 4, 2048, 1024, 32
HALF = N // 2            # rows per core
CLAMP = 10.0
N_CORES = 8
P = 128                  # partitions
JT = 512                 # moving free dim per matmul
NJ = N // JT             # 4 column chunks
NKC = D // P             # 8 contraction chunks for G
R1, R2 = 32, 64          # augmentation rows (must be multiples of 32)

# H/W dtype for the G matmul.  fp8 halves the ht DMA (1 MiB vs 2 MiB bf16);
# W is pre-scaled by WSCALE host-side so its ~1e-3 entries stay in fp8's
# normal range, and the scale is divided back out in the PSUM->SBUF copy.
# fp8 also enables DoubleRow matmuls (256-deep contraction per instruction).
HT8 = os.environ.get("KERNEL_HT8", "1") != "0"
H_DT = F8 if HT8 else BF16
WSCALE = 1024.0 if HT8 else 1.0
_nc_cache: dict = {}


def _build_nc(alpha: float, beta: float, loop_reps: int | None = None) -> "bass.Bass":
    # Bacc (not raw Bass): its finalize() runs the legalization passes that
    # split multi-sem waits (PE instructions have a single wait slot).
    nc = bacc.Bacc(None, num_devices=N_CORES)
    ht = nc.dram_tensor("ht", [D, N], H_DT, kind="ExternalInput")
    # wt is host-prepacked into the [partition, chunk*K] SBUF layout so the
    # DMA is one contiguous 256 B run per partition.
    wt = nc.dram_tensor("wt", [P, NKC * K], H_DT, kind="ExternalInput")
    bp_in = nc.dram_tensor("bprev", [HALF, N], BF16, kind="ExternalInput")
    out = nc.dram_tensor("out", [HALF, N], BF16, kind="ExternalOutput")

    with TileContext(nc) as tc:
        # Pools are shared across benchmark reps so PSUM/SBUF slot reuse
        # carries proper cross-rep dependencies.
        # PSUM budget: one pool of 3*[128,1024] = 6 banks.  The G phase
        # borrows two of these tiles (all four j-chunk PSUMs live at once, so
        # no recycle stalls in the head); dp=3 lets dist tile k+1's matmuls
        # start before tile k's PSUM consumers finish.
        with (
            tc.tile_pool(name="persist", bufs=1) as persist,
            tc.tile_pool(name="dpsum", bufs=3, space="PSUM") as dp,
            tc.tile_pool(name="qpsum", bufs=2, space="PSUM") as qp,
            tc.tile_pool(name="bpool", bufs=8) as bpool,
            tc.tile_pool(name="rpool", bufs=8) as rpool,
            tc.tile_pool(name="tpool", bufs=4) as tpool,
            tc.tile_pool(name="opool", bufs=8) as opool,
        ):
            pools = dict(
                persist=persist, dp=dp, qp=qp, bpool=bpool,
                rpool=rpool, tpool=tpool, opool=opool,
            )
            for _ in range(loop_reps or 1):
                _emit_body(nc, tc, pools, ht, wt, bp_in, out, alpha, beta)
    if not nc.is_finalized():
        nc.finalize()
    return nc


def _emit_body(nc, tc, pools, ht, wt, bp_in, out, alpha: float, beta: float):
    nb = -float(beta)
    persist, dp, qp = pools["persist"], pools["dp"], pools["qp"]
    bpool, rpool, tpool, opool = (
        pools["bpool"], pools["rpool"], pools["tpool"], pools["opool"]
    )

    # ---- DMA issue order: ht then all B_prev tiles on the sync queue, the
    # small wt via SWDGE on the Pool queue (so it neither delays ht's issue
    # nor its transfer; transfer requests hit the shared DMA engines in
    # roughly this order and the G phase is never starved by prefetch).
    wt_sb = persist.tile([P, NKC, K], H_DT, tag="wt_sb")
    nc.gpsimd.dma_start(out=wt_sb[:], in_=wt.rearrange("p (c k) -> p c k", c=NKC))

    ht_sb = persist.tile([P, NKC, N], H_DT, tag="ht_sb")
    htr = ht.rearrange("(c p) j -> p c j", p=P)
    for g in range(4):
        nc.sync.dma_start(
            out=ht_sb[:, 2 * g : 2 * g + 2, :], in_=htr[:, 2 * g : 2 * g + 2, :]
        )

    bts = []
    for it in range(HALF // P):
        btile = bpool.tile([P, N], BF16, tag="bt")
        nc.sync.dma_start(out=btile[:], in_=bp_in[it * P : (it + 1) * P, :])
        bts.append(btile)

    # ---- one-time constants / zero padding (Pool engine; off the DMA path)
    ones_sb = persist.tile([K, 1], BF16, tag="ones_sb")
    nc.gpsimd.memset(ones_sb[:], 1.0)
    # rhs_aug rows: 0..31 = -2*G_j | 32 = 1 | 64 = |G_j|^2 ; rest zero.
    # lhs_aug rows: 0..31 = -b*G_i | 32 = -b*|G_i|^2 | 64 = -b ; rest zero.
    rhs_aug = persist.tile([P, N], BF16, tag="rhs_aug")
    lhs_aug = persist.tile([P, HALF], BF16, tag="lhs_aug")
    gsq_in = persist.tile([K, N], BF16, tag="gsq_in")
    nc.gpsimd.memset(rhs_aug[:], 0.0)
    nc.gpsimd.memset(lhs_aug[:], 0.0)
    nc.gpsimd.memset(rhs_aug[R1 : R1 + 1, :], 1.0)
    nc.gpsimd.memset(lhs_aug[R2 : R2 + 1, :], nb)

    # Warm the ACT function table (Copy+Relu) at t~0 on a 1-element dummy so
    # the 1283 ns LoadActFuncSet overlaps the ht DMA instead of stalling the
    # first G-phase copy.
    warm = persist.tile([1, 1], BF16, tag="warm")
    nc.gpsimd.memset(warm[:], 0.0)
    nc.scalar.activation(warm[:], warm[:], AF.Relu)

    # ---------------- G phase (full G[b], computed locally) ----------------
    # psum = WSCALE * G^T[k, js]; the PSUM->SBUF copy descales and applies
    # the -2.  fp8 DoubleRow contracts a kc-pair (256 rows) per matmul.
    # PSUM->SBUF copies alternate ACT/DVE per chunk so neither engine
    # serializes the G tail.
    # Two borrowed dist-PSUM tiles hold all four j-chunk G PSUMs at once
    # (G in partitions 0..31, the gsq row-sum in partition 32 of the same
    # banks), so the G tail has no PSUM-recycle stalls.
    pga = dp.tile([P, N // 2], F32, tag="pd")
    pgb = dp.tile([P, N // 2], F32, tag="pd")
    pgs = [pga, pgb]
    # All 16 G matmuls are emitted before any PSUM consumer: PE runs in
    # program order, so interleaving the (ACT/DVE-gated) gsq ones-matmuls
    # here would head-of-line-block the later j-chunks' G matmuls.
    for jc in range(NJ):
        js = slice(jc * JT, (jc + 1) * JT)
        pg = pgs[jc // 2][:, (jc % 2) * JT : (jc % 2 + 1) * JT]
        if HT8:
            for g in range(4):
                nc.tensor.matmul(
                    pg[0:K, :],
                    wt_sb[:, 2 * g : 2 * g + 2, :],
                    ht_sb[:, 2 * g : 2 * g + 2, js],
                    start=(g == 0),
                    stop=(g == 3),
                    perf_mode=mybir.MatmulPerfMode.DoubleRow,
                )
        else:
            for kc in range(NKC):
                nc.tensor.matmul(
                    pg[0:K, :],
                    wt_sb[:, kc, :],
                    ht_sb[:, kc, js],
                    start=(kc == 0),
                    stop=(kc == NKC - 1),
                )
    # Per-chunk consumer chains, hand-ordered to steer the list scheduler:
    # the j3 chain (psum -> square -> ones-mm -> R2) is the lhs barrier's
    # critical path, so its square reads PSUM directly (skipping the copy
    # hop) and is emitted before copy-j3; its ones-mm and R2 are emitted
    # before the other chunks'.  copies: ACT (even) / DVE (odd); squares:
    # j0 Pool from SBUF, j1/j2 ACT from PSUM, j3 DVE from PSUM; R2 descale:
    # j0/j2 ACT, j1/j3 DVE.  gsq scale varies per source (see r2_scale).
    def _copy(jc):
        js = slice(jc * JT, (jc + 1) * JT)
        pg = pgs[jc // 2][:, (jc % 2) * JT : (jc % 2 + 1) * JT]
        if jc % 2 == 0:
            nc.scalar.activation(
                rhs_aug[0:K, js], pg[0:K, :], AF.Copy, scale=-2.0 / WSCALE
            )
        else:
            nc.vector.tensor_scalar_mul(rhs_aug[0:K, js], pg[0:K, :], -2.0 / WSCALE)

    def _square(jc):
        js = slice(jc * JT, (jc + 1) * JT)
        pg = pgs[jc // 2][:, (jc % 2) * JT : (jc % 2 + 1) * JT]
        if jc == 0:
            nc.gpsimd.tensor_tensor(
                gsq_in[:, js], rhs_aug[0:K, js], rhs_aug[0:K, js], ALU.mult
            )
        elif jc == 1:
            nc.vector.tensor_tensor(
                gsq_in[:, js], rhs_aug[0:K, js], rhs_aug[0:K, js], ALU.mult
            )
        elif jc == 3:
            # TensorTensor may read only one PSUM operand on hardware, so
            # j3 squares the SBUF copy (-2G)^2 = 4 G^2 instead.
            nc.vector.tensor_tensor(
                gsq_in[:, js], rhs_aug[0:K, js], rhs_aug[0:K, js], ALU.mult
            )
        else:
            nc.scalar.activation(
                gsq_in[:, js], pg[0:K, :], AF.Square, scale=1.0 / WSCALE
            )

    pq0 = qp.tile([1, JT], F32, tag="pq")
    pq1 = qp.tile([1, JT], F32, tag="pq")
    pqs = [pq0[:], pq1[:], pga[K : K + 1, 0:JT], pga[K : K + 1, JT : 2 * JT]]

    def _r2(jc):
        js = slice(jc * JT, (jc + 1) * JT)
        nc.tensor.matmul(pqs[jc], ones_sb[:], gsq_in[:, js], start=True, stop=True)
        r2_scale = [0.25, 0.25, 1.0, 0.25][jc]
        if jc == 1:
            nc.vector.tensor_scalar_mul(rhs_aug[R2 : R2 + 1, js], pqs[jc], r2_scale)
        else:
            nc.scalar.activation(
                rhs_aug[R2 : R2 + 1, js], pqs[jc], AF.Copy, scale=r2_scale
            )

    # The host rotates each core's columns so its own row-half is always
    # columns [0:HALF): lhs_aug depends statically on the j0/j1 chains only,
    # so the dist phase starts while j2/j3's gsq work is still in flight.
    for jc in (0, 1):
        _copy(jc)
        _square(jc)
        _r2(jc)
    nc.vector.tensor_scalar_mul(
        lhs_aug[0:K, 0:HALF], rhs_aug[0:K, 0:HALF], float(beta) / 2.0
    )
    nc.vector.tensor_scalar_mul(
        lhs_aug[R1 : R1 + 1, 0:HALF], rhs_aug[R2 : R2 + 1, 0:HALF], nb
    )
    # (j2/j3 consumer chains are emitted inside the dist phase below, after
    # tile 0's first half, so PE doesn't head-of-line-block on their pq mms.)

    # ---------------- dist + EMA phase ----------------
    # Tile 0's first half is emitted between the j0/j1 and j2/j3 consumer
    # chains (see above); every (it, hh) piece is otherwise identical.
    tts = {}

    def _dist(it, hh):
        isl = slice(it * P, (it + 1) * P)
        bt = bts[it]
        if it not in tts:
            if alpha != 1.0:
                nc.vector.tensor_scalar_mul(bt[:], bt[:], float(alpha))
            tt = tpool.tile([P, N], BF16, tag="tt")
            tts[it] = tt
        tt = tts[it]
        if True:
            hs = slice(hh * (N // 2), (hh + 1) * (N // 2))
            pd = dp.tile([P, N // 2], F32, tag="pd")
            for jc2 in range(2):
                jl = slice(jc2 * JT, (jc2 + 1) * JT)
                jg = slice(hh * (N // 2) + jc2 * JT, hh * (N // 2) + (jc2 + 1) * JT)
                nc.tensor.matmul(
                    pd[:, jl], lhs_aug[:, isl], rhs_aug[:, jg], start=True, stop=True
                )
            # tt = bt + min(psum, 0) = bt - beta*max(dist, 0).  Pool cannot
            # read PSUM on hardware, so the 2048 PSUM columns are consumed
            # ACT:DVE at roughly 5:1 -- ACT Relu(-psum) on half 0 plus 704
            # columns of half 1 (DVE subtracts bt from each), a DVE STT
            # straight from PSUM on the remaining 320 -- and the clamps
            # split Pool (half 0) / DVE (half 1), balancing all three
            # engines under the ~1.9us/tile pipeline cadence.
            if hh == 0:
                r16 = rpool.tile([P, N // 2], BF16, tag="r16")
                nc.scalar.activation(r16[:], pd[:], AF.Relu, scale=-1.0)
                nc.vector.tensor_tensor(tt[:, hs], bt[:, hs], r16[:], ALU.subtract)
            else:
                RW = 704  # ACT-relu'd columns of half 1; DVE STTs the rest
                r16 = rpool.tile([P, N // 2], BF16, tag="r16")
                nc.scalar.activation(r16[:, 0:RW], pd[:, 0:RW], AF.Relu, scale=-1.0)
                nc.vector.tensor_tensor(
                    tt[:, N // 2 : N // 2 + RW], bt[:, N // 2 : N // 2 + RW],
                    r16[:, 0:RW], ALU.subtract,
                )
                nc.vector.scalar_tensor_tensor(
                    tt[:, N // 2 + RW : N], pd[:, RW : 2 * JT], 0.0,
                    bt[:, N // 2 + RW : N], ALU.min, ALU.add,
                )
            oth = opool.tile([P, N // 2], BF16, tag="oth")
            if hh == 0:
                nc.gpsimd.tensor_scalar(
                    oth[:], tt[:, hs], CLAMP, -CLAMP, ALU.min, ALU.max
                )
            else:
                nc.vector.tensor_scalar(
                    oth[:], tt[:, hs], CLAMP, -CLAMP, ALU.min, ALU.max
                )
            nc.sync.dma_start(out=out[isl, hs], in_=oth[:])

    _dist(0, 0)
    for jc in (2, 3):
        _copy(jc)
        _square(jc)
        _r2(jc)
    _dist(0, 1)
    for it in range(1, HALF // P):
        _dist(it, 0)
        _dist(it, 1)


def _get_nc(alpha: float, beta: float) -> "bass.Bass":
    key = (alpha, beta)
    if key not in _nc_cache:
        _nc_cache[key] = _build_nc(alpha, beta)
    return _nc_cache[key]


def _make_in_maps(H, B_prev, W):
    h_np = mybir.dt.np(H_DT)
    # wt prepacked to the SBUF layout: wt_host[p, c*K+k] = WSCALE*W^T[c*P+p, k]
    wtt = (W.T * WSCALE).reshape(NKC, P, K)  # [c, p, k]
    wt_host = np.ascontiguousarray(wtt.transpose(1, 0, 2).reshape(P, NKC * K)).astype(
        h_np
    )
    bf_np = mybir.dt.np(BF16)
    in_maps = []
    for c in range(N_CORES):
        bidx, h = divmod(c, 2)
        htb = np.ascontiguousarray(H[bidx].T).astype(h_np)  # [1024, 2048]
        bp = B_prev[bidx, h * HALF : (h + 1) * HALF, :].astype(bf_np)
        if h == 1:
            # rotate columns so this core's own rows come first
            htb = np.concatenate([htb[:, HALF:], htb[:, :HALF]], axis=1)
            bp = np.concatenate([bp[:, HALF:], bp[:, :HALF]], axis=1)
        in_maps.append(
            {
                "ht": np.ascontiguousarray(htb),
                "wt": wt_host,
                "bprev": np.ascontiguousarray(bp),
            }
        )
    return in_maps


def _assemble(results) -> np.ndarray:
    out = np.empty((B, N, N), np.float32)
    for c in range(N_CORES):
        bidx, h = divmod(c, 2)
        r = results[c]["out"].astype(np.float32)
        if h == 1:
            r = np.concatenate([r[:, HALF:], r[:, :HALF]], axis=1)
        out[bidx, h * HALF : (h + 1) * HALF, :] = r
    return out


def _run(H, B_prev, W, alpha, beta, **rbk_kwargs):
    H = np.ascontiguousarray(np.asarray(H, dtype=np.float32))
    B_prev = np.ascontiguousarray(np.asarray(B_prev, dtype=np.float32))
    W = np.ascontiguousarray(np.asarray(W, dtype=np.float32))
    nc = _get_nc(float(alpha), float(beta))
    in_maps = _make_in_maps(H, B_prev, W)
    res = run_bass_kernel_spmd(nc, in_maps, list(range(N_CORES)), **rbk_kwargs)
    return _assemble(res.results), res


def kernel(H, B_prev, W, alpha, beta) -> np.ndarray:
    out, _ = _run(H, B_prev, W, alpha, beta)
    return out


# revision 62
# speedup vs baseline: 2.4589x; 1.0141x over previous
"""Trainium2 Bass kernel for nn_MetricBiasUpdater.

Computes, for H [4,2048,1024], B_prev [4,2048,2048], W [32,1024]:
    G    = H @ W.T                                   [4,2048,32]
    dist = |G_i|^2 + |G_j|^2 - 2 G_i.G_j             [4,2048,2048]
    out  = clip(alpha*B_prev - beta*max(dist,0), -10, 10)

Sharding: 8 cores = (batch b, row-half h).  Core (b,h) computes output rows
[h*1024,(h+1)*1024) of batch b for all 2048 columns.  Each core reads the
full H[b]^T (in a reduced dtype) and computes the full G[b] locally -- no
collectives; the redundant read is cheaper than the 3-hop DRAM round-trip
latency of a pair exchange.  The host rotates each core's columns so its
own row-half is always columns [0:1024): the lhs operand build then depends
statically on only the first two G column chunks, and the first dist-phase
matmuls overlap the remaining chunks' gsq work.

Precision: the harness tolerance is rel_err < 2e-2.  B_prev is read and the
output is written in bf16 (host-side cast, ~0.2% rms each), halving the two
dominant HBM streams.  H/W enter the G matmul in fp8-e4m3 with W pre-scaled
by 1024 (descaled exactly in the PSUM->SBUF copy); G only feeds the dist
term, which contributes ~0.3% of the output magnitude, so fp8's ~3% error
on G is negligible.  Measured rel err ~3.5e-3 overall.

On-core algorithm: one augmented matmul produces -beta*dist directly:
    lhsT = [-beta*G_i; -beta*|G_i|^2; -beta]   (K padded 34 -> 128, zeros)
    rhs  = [-2*G_j; 1; |G_j|^2]
    psum[i,j] = -beta*dist[i,j]
then per 128-row i-tile the PSUM is turned into clip(bt - beta*max(dist,0))
with the work spread over ACT (Relu from PSUM), DVE (bf16 subtract, STT,
clamp) and Pool (bf16 clamp), so the pipeline stays close to the DMA drain
rate (1 MiB/tile at 360 B/ns).

All load/store DMAs are issued on the sync (SP) queue in priority order --
ht x4, B_prev x8, then stores -- (wt goes via SWDGE on the Pool queue) so
the head (ht -> G -> augmented operands) is never starved by prefetch and
the DMA engines stay saturated end to end.

SBUF partition-offset rule: sub-128-partition accesses must start at a
multiple of 32, so the two augmentation rows live at partitions 32 and 64
(rows 33..63 and 65..127 stay zero and contribute nothing to the matmul).
"""

import os
import sys

# The bass runtime drives the NeuronCores through the jax "axon" PJRT
# platform.  If a caller pinned JAX_PLATFORMS to cpu (common for running
# the pure-jax reference), undo that before jax is first imported.
if "jax" not in sys.modules:
    _jp = os.environ.get("JAX_PLATFORMS")
    if _jp is not None and "axon" not in _jp and "neuron" not in _jp:
        del os.environ["JAX_PLATFORMS"]

sys.path.insert(0, "/opt/trn_rl_repo")

import numpy as np

import concourse.bass as bass
import concourse.bacc as bacc
import concourse.mybir as mybir
from concourse.tile import TileContext
from concourse.bass_utils import run_bass_kernel_spmd

F32 = mybir.dt.float32
BF16 = mybir.dt.bfloat16
F8 = mybir.dt.float8e4
AF = mybir.ActivationFunctionType
ALU = mybir.AluOpType

B, N, D, K = 4, 2048, 1024, 32
HALF = N // 2            # rows per core
CLAMP = 10.0
N_CORES = 8
P = 128                  # partitions
JT = 512                 # moving free dim per matmul
NJ = N // JT             # 4 column chunks
NKC = D // P             # 8 contraction chunks for G
R1, R2 = 32, 64          # augmentation rows (must be multiples of 32)

# H/W dtype for the G matmul.  fp8 halves the ht DMA (1 MiB vs 2 MiB bf16);
# W is pre-scaled by WSCALE host-side so its ~1e-3 entries stay in fp8's
# normal range, and the scale is divided back out in the PSUM->SBUF copy.
# fp8 also enables DoubleRow matmuls (256-deep contraction per instruction).
HT8 = os.environ.get("KERNEL_HT8", "1") != "0"
H_DT = F8 if HT8 else BF16
WSCALE = 1024.0 if HT8 else 1.0
_nc_cache: dict = {}


def _build_nc(alpha: float, beta: float, loop_reps: int | None = None) -> "bass.Bass":
    # Bacc (not raw Bass): its finalize() runs the legalization passes that
    # split multi-sem waits (PE instructions have a single wait slot).
    nc = bacc.Bacc(None, num_devices=N_CORES)
    ht = nc.dram_tensor("ht", [D, N], H_DT, kind="ExternalInput")
    # wt is host-prepacked into the [partition, chunk*K] SBUF layout so the
    # DMA is one contiguous 256 B run per partition.
    wt = nc.dram_tensor("wt", [P, NKC * K], H_DT, kind="ExternalInput")
    bp_in = nc.dram_tensor("bprev", [HALF, N], BF16, kind="ExternalInput")
    out = nc.dram_tensor("out", [HALF, N], BF16, kind="ExternalOutput")

    with TileContext(nc) as tc:
        # Pools are shared across benchmark reps so PSUM/SBUF slot reuse
        # carries proper cross-rep dependencies.
        # PSUM budget: one pool of 3*[128,1024] = 6 banks.  The G phase
        # borrows two of these tiles (all four j-chunk PSUMs live at once, so
        # no recycle stalls in the head); dp=3 lets dist tile k+1's matmuls
        # start before tile k's PSUM consumers finish.
        with (
            tc.tile_pool(name="persist", bufs=1) as persist,
            tc.tile_pool(name="dpsum", bufs=3, space="PSUM") as dp,
            tc.tile_pool(name="qpsum", bufs=2, space="PSUM") as qp,
            tc.tile_pool(name="bpool", bufs=8) as bpool,
            tc.tile_pool(name="rpool", bufs=8) as rpool,
            tc.tile_pool(name="tpool", bufs=4) as tpool,
            tc.tile_pool(name="opool", bufs=8) as opool,
        ):
            pools = dict(
                persist=persist, dp=dp, qp=qp, bpool=bpool,
                rpool=rpool, tpool=tpool, opool=opool,
            )
            for _ in range(loop_reps or 1):
                _emit_body(nc, tc, pools, ht, wt, bp_in, out, alpha, beta)
    if not nc.is_finalized():
        nc.finalize()
    return nc


def _emit_body(nc, tc, pools, ht, wt, bp_in, out, alpha: float, beta: float):
    nb = -float(beta)
    persist, dp, qp = pools["persist"], pools["dp"], pools["qp"]
    bpool, rpool, tpool, opool = (
        pools["bpool"], pools["rpool"], pools["tpool"], pools["opool"]
    )

    # ---- DMA issue order: ht then all B_prev tiles on the sync queue, the
    # small wt via SWDGE on the Pool queue (so it neither delays ht's issue
    # nor its transfer; transfer requests hit the shared DMA engines in
    # roughly this order and the G phase is never starved by prefetch).
    wt_sb = persist.tile([P, NKC, K], H_DT, tag="wt_sb")
    nc.gpsimd.dma_start(out=wt_sb[:], in_=wt.rearrange("p (c k) -> p c k", c=NKC))

    ht_sb = persist.tile([P, NKC, N], H_DT, tag="ht_sb")
    htr = ht.rearrange("(c p) j -> p c j", p=P)
    for g in range(4):
        nc.sync.dma_start(
            out=ht_sb[:, 2 * g : 2 * g + 2, :], in_=htr[:, 2 * g : 2 * g + 2, :]
        )

    bts = []
    for it in range(HALF // P):
        btile = bpool.tile([P, N], BF16, tag="bt")
        nc.sync.dma_start(out=btile[:], in_=bp_in[it * P : (it + 1) * P, :])
        bts.append(btile)

    # ---- one-time constants / zero padding (Pool engine; off the DMA path)
    ones_sb = persist.tile([K, 1], BF16, tag="ones_sb")
    nc.gpsimd.memset(ones_sb[:], 1.0)
    # rhs_aug rows: 0..31 = -2*G_j | 32 = 1 | 64 = |G_j|^2 ; rest zero.
    # lhs_aug rows: 0..31 = -b*G_i | 32 = -b*|G_i|^2 | 64 = -b ; rest zero.
    rhs_aug = persist.tile([P, N], BF16, tag="rhs_aug")
    lhs_aug = persist.tile([P, HALF], BF16, tag="lhs_aug")
    gsq_in = persist.tile([K, N], BF16, tag="gsq_in")
    nc.gpsimd.memset(rhs_aug[:], 0.0)
    nc.gpsimd.memset(lhs_aug[:], 0.0)
    nc.gpsimd.memset(rhs_aug[R1 : R1 + 1, :], 1.0)
    nc.gpsimd.memset(lhs_aug[R2 : R2 + 1, :], nb)

    # Warm the ACT function table (Copy+Relu) at t~0 on a 1-element dummy so
    # the 1283 ns LoadActFuncSet overlaps the ht DMA instead of stalling the
    # first G-phase copy.
    warm = persist.tile([1, 1], BF16, tag="warm")
    nc.gpsimd.memset(warm[:], 0.0)
    nc.scalar.activation(warm[:], warm[:], AF.Relu)

    # ---------------- G phase (full G[b], computed locally) ----------------
    # psum = WSCALE * G^T[k, js]; the PSUM->SBUF copy descales and applies
    # the -2.  fp8 DoubleRow contracts a kc-pair (256 rows) per matmul.
    # PSUM->SBUF copies alternate ACT/DVE per chunk so neither engine
    # serializes the G tail.
    # Two borrowed dist-PSUM tiles hold all four j-chunk G PSUMs at once
    # (G in partitions 0..31, the gsq row-sum in partition 32 of the same
    # banks), so the G tail has no PSUM-recycle stalls.
    pga = dp.tile([P, N // 2], F32, tag="pd")
    pgb = dp.tile([P, N // 2], F32, tag="pd")
    pgs = [pga, pgb]
    # All 16 G matmuls are emitted before any PSUM consumer: PE runs in
    # program order, so interleaving the (ACT/DVE-gated) gsq ones-matmuls
    # here would head-of-line-block the later j-chunks' G matmuls.  The
    # contraction (g) loop is outermost and j1/j0 lead, so once the last ht
    # chunk lands PE finishes the j1/j0 PSUMs first -- they root the lhs
    # barrier chain; j2/j3 finish last and their consumers have slack.
    # Interleaved accumulation groups write different PSUM banks, which the
    # hardware tracks independently (hence skip_group_check).
    jorder = (1, 0, 2, 3)
    if HT8:
        for g in range(4):
            for jc in jorder:
                js = slice(jc * JT, (jc + 1) * JT)
                pg = pgs[jc // 2][:, (jc % 2) * JT : (jc % 2 + 1) * JT]
                nc.tensor.matmul(
                    pg[0:K, :],
                    wt_sb[:, 2 * g : 2 * g + 2, :],
                    ht_sb[:, 2 * g : 2 * g + 2, js],
                    start=(g == 0),
                    stop=(g == 3),
                    perf_mode=mybir.MatmulPerfMode.DoubleRow,
                    skip_group_check=True,
                )
    else:
        for kc in range(NKC):
            for jc in jorder:
                js = slice(jc * JT, (jc + 1) * JT)
                pg = pgs[jc // 2][:, (jc % 2) * JT : (jc % 2 + 1) * JT]
                nc.tensor.matmul(
                    pg[0:K, :],
                    wt_sb[:, kc, :],
                    ht_sb[:, kc, js],
                    start=(kc == 0),
                    stop=(kc == NKC - 1),
                    skip_group_check=True,
                )
    # Per-chunk consumer chains, hand-ordered to steer the list scheduler:
    # the j3 chain (psum -> square -> ones-mm -> R2) is the lhs barrier's
    # critical path, so its square reads PSUM directly (skipping the copy
    # hop) and is emitted before copy-j3; its ones-mm and R2 are emitted
    # before the other chunks'.  copies: ACT (even) / DVE (odd); squares:
    # j0 Pool from SBUF, j1/j2 ACT from PSUM, j3 DVE from PSUM; R2 descale:
    # j0/j2 ACT, j1/j3 DVE.  gsq scale varies per source (see r2_scale).
    def _copy(jc):
        js = slice(jc * JT, (jc + 1) * JT)
        pg = pgs[jc // 2][:, (jc % 2) * JT : (jc % 2 + 1) * JT]
        if jc % 2 == 0:
            nc.scalar.activation(
                rhs_aug[0:K, js], pg[0:K, :], AF.Copy, scale=-2.0 / WSCALE
            )
        else:
            nc.vector.tensor_scalar_mul(rhs_aug[0:K, js], pg[0:K, :], -2.0 / WSCALE)

    def _square(jc):
        js = slice(jc * JT, (jc + 1) * JT)
        pg = pgs[jc // 2][:, (jc % 2) * JT : (jc % 2 + 1) * JT]
        if jc == 0:
            nc.gpsimd.tensor_tensor(
                gsq_in[:, js], rhs_aug[0:K, js], rhs_aug[0:K, js], ALU.mult
            )
        elif jc == 1:
            nc.vector.tensor_tensor(
                gsq_in[:, js], rhs_aug[0:K, js], rhs_aug[0:K, js], ALU.mult
            )
        elif jc == 3:
            # TensorTensor may read only one PSUM operand on hardware, so
            # j3 squares the SBUF copy (-2G)^2 = 4 G^2 instead.
            nc.vector.tensor_tensor(
                gsq_in[:, js], rhs_aug[0:K, js], rhs_aug[0:K, js], ALU.mult
            )
        else:
            nc.scalar.activation(
                gsq_in[:, js], pg[0:K, :], AF.Square, scale=1.0 / WSCALE
            )

    pq0 = qp.tile([1, JT], F32, tag="pq")
    pq1 = qp.tile([1, JT], F32, tag="pq")
    pqs = [pq0[:], pq1[:], pga[K : K + 1, 0:JT], pga[K : K + 1, JT : 2 * JT]]

    def _r2(jc):
        js = slice(jc * JT, (jc + 1) * JT)
        nc.tensor.matmul(pqs[jc], ones_sb[:], gsq_in[:, js], start=True, stop=True)
        r2_scale = [0.25, 0.25, 1.0, 0.25][jc]
        if jc == 1:
            nc.vector.tensor_scalar_mul(rhs_aug[R2 : R2 + 1, js], pqs[jc], r2_scale)
        else:
            nc.scalar.activation(
                rhs_aug[R2 : R2 + 1, js], pqs[jc], AF.Copy, scale=r2_scale
            )

    # The host rotates each core's columns so its own row-half is always
    # columns [0:HALF): lhs_aug depends statically on the j0/j1 chains only,
    # so the dist phase starts while j2/j3's gsq work is still in flight.
    for jc in (0, 1):
        _copy(jc)
        _square(jc)
        _r2(jc)
    nc.vector.tensor_scalar_mul(
        lhs_aug[0:K, 0:HALF], rhs_aug[0:K, 0:HALF], float(beta) / 2.0
    )
    nc.vector.tensor_scalar_mul(
        lhs_aug[R1 : R1 + 1, 0:HALF], rhs_aug[R2 : R2 + 1, 0:HALF], nb
    )
    # (j2/j3 consumer chains are emitted inside the dist phase below, after
    # tile 0's first half, so PE doesn't head-of-line-block on their pq mms.)

    # ---------------- dist + EMA phase ----------------
    # Tile 0's first half is emitted between the j0/j1 and j2/j3 consumer
    # chains (see above); every (it, hh) piece is otherwise identical.
    tts = {}

    def _dist(it, hh):
        isl = slice(it * P, (it + 1) * P)
        bt = bts[it]
        if it not in tts:
            if alpha != 1.0:
                nc.vector.tensor_scalar_mul(bt[:], bt[:], float(alpha))
            tt = tpool.tile([P, N], BF16, tag="tt")
            tts[it] = tt
        tt = tts[it]
        if True:
            hs = slice(hh * (N // 2), (hh + 1) * (N // 2))
            pd = dp.tile([P, N // 2], F32, tag="pd")
            for jc2 in range(2):
                jl = slice(jc2 * JT, (jc2 + 1) * JT)
                jg = slice(hh * (N // 2) + jc2 * JT, hh * (N // 2) + (jc2 + 1) * JT)
                nc.tensor.matmul(
                    pd[:, jl], lhs_aug[:, isl], rhs_aug[:, jg], start=True, stop=True
                )
            # tt = bt + min(psum, 0) = bt - beta*max(dist, 0).  Pool cannot
            # read PSUM on hardware, so the 2048 PSUM columns are consumed
            # ACT:DVE at roughly 5:1 -- ACT Relu(-psum) on half 0 plus 704
            # columns of half 1 (DVE subtracts bt from each), a DVE STT
            # straight from PSUM on the remaining 320 -- and the clamps
            # split Pool (half 0) / DVE (half 1), balancing all three
            # engines under the ~1.9us/tile pipeline cadence.
            if hh == 0:
                r16 = rpool.tile([P, N // 2], BF16, tag="r16")
                nc.scalar.activation(r16[:], pd[:], AF.Relu, scale=-1.0)
                nc.vector.tensor_tensor(tt[:, hs], bt[:, hs], r16[:], ALU.subtract)
            else:
                RW = 704  # ACT-relu'd columns of half 1; DVE STTs the rest
                r16 = rpool.tile([P, N // 2], BF16, tag="r16")
                nc.scalar.activation(r16[:, 0:RW], pd[:, 0:RW], AF.Relu, scale=-1.0)
                nc.vector.tensor_tensor(
                    tt[:, N // 2 : N // 2 + RW], bt[:, N // 2 : N // 2 + RW],
                    r16[:, 0:RW], ALU.subtract,
                )
                nc.vector.scalar_tensor_tensor(
                    tt[:, N // 2 + RW : N], pd[:, RW : 2 * JT], 0.0,
                    bt[:, N // 2 + RW : N], ALU.min, ALU.add,
                )
            oth = opool.tile([P, N // 2], BF16, tag="oth")
            if hh == 0:
                nc.gpsimd.tensor_scalar(
                    oth[:], tt[:, hs], CLAMP, -CLAMP, ALU.min, ALU.max
                )
            else:
                nc.vector.tensor_scalar(
                    oth[:], tt[:, hs], CLAMP, -CLAMP, ALU.min, ALU.max
                )
            nc.sync.dma_start(out=out[isl, hs], in_=oth[:])

    _dist(0, 0)
    for jc in (2, 3):
        _copy(jc)
        _square(jc)
        _r2(jc)
    _dist(0, 1)
    for it in range(1, HALF // P):
        _dist(it, 0)
        _dist(it, 1)


def _get_nc(alpha: float, beta: float) -> "bass.Bass":
    key = (alpha, beta)
    if key not in _nc_cache:
        _nc_cache[key] = _build_nc(alpha, beta)
    return _nc_cache[key]


def _make_in_maps(H, B_prev, W):
    h_np = mybir.dt.np(H_DT)
    # wt prepacked to the SBUF layout: wt_host[p, c*K+k] = WSCALE*W^T[c*P+p, k]
    wtt = (W.T * WSCALE).reshape(NKC, P, K)  # [c, p, k]
    wt_host = np.ascontiguousarray(wtt.transpose(1, 0, 2).reshape(P, NKC * K)).astype(
        h_np
    )
    bf_np = mybir.dt.np(BF16)
    in_maps = []
    for c in range(N_CORES):
        bidx, h = divmod(c, 2)
        htb = np.ascontiguousarray(H[bidx].T).astype(h_np)  # [1024, 2048]
        bp = B_prev[bidx, h * HALF : (h + 1) * HALF, :].astype(bf_np)
        if h == 1:
            # rotate columns so this core's own rows come first
            htb = np.concatenate([htb[:, HALF:], htb[:, :HALF]], axis=1)
            bp = np.concatenate([bp[:, HALF:], bp[:, :HALF]], axis=1)
        in_maps.append(
            {
                "ht": np.ascontiguousarray(htb),
                "wt": wt_host,
                "bprev": np.ascontiguousarray(bp),
            }
        )
    return in_maps


def _assemble(results) -> np.ndarray:
    out = np.empty((B, N, N), np.float32)
    for c in range(N_CORES):
        bidx, h = divmod(c, 2)
        r = results[c]["out"].astype(np.float32)
        if h == 1:
            r = np.concatenate([r[:, HALF:], r[:, :HALF]], axis=1)
        out[bidx, h * HALF : (h + 1) * HALF, :] = r
    return out


def _run(H, B_prev, W, alpha, beta, **rbk_kwargs):
    H = np.ascontiguousarray(np.asarray(H, dtype=np.float32))
    B_prev = np.ascontiguousarray(np.asarray(B_prev, dtype=np.float32))
    W = np.ascontiguousarray(np.asarray(W, dtype=np.float32))
    nc = _get_nc(float(alpha), float(beta))
    in_maps = _make_in_maps(H, B_prev, W)
    res = run_bass_kernel_spmd(nc, in_maps, list(range(N_CORES)), **rbk_kwargs)
    return _assemble(res.results), res


def kernel(H, B_prev, W, alpha, beta) -> np.ndarray:
    out, _ = _run(H, B_prev, W, alpha, beta)
    return out


# revision 75
# speedup vs baseline: 2.4911x; 1.0131x over previous
"""Trainium2 Bass kernel for nn_MetricBiasUpdater.

Computes, for H [4,2048,1024], B_prev [4,2048,2048], W [32,1024]:
    G    = H @ W.T                                   [4,2048,32]
    dist = |G_i|^2 + |G_j|^2 - 2 G_i.G_j             [4,2048,2048]
    out  = clip(alpha*B_prev - beta*max(dist,0), -10, 10)

Sharding: 8 cores = (batch b, row-half h).  Core (b,h) computes output rows
[h*1024,(h+1)*1024) of batch b for all 2048 columns.  Each core reads the
full H[b]^T (in a reduced dtype) and computes the full G[b] locally -- no
collectives; the redundant read is cheaper than the 3-hop DRAM round-trip
latency of a pair exchange.  The host rotates each core's columns so its
own row-half is always columns [0:1024): the lhs operand build then depends
statically on only the first two G column chunks, and the first dist-phase
matmuls overlap the remaining chunks' gsq work.

Precision: the harness tolerance is rel_err < 2e-2.  B_prev is read and the
output is written in bf16 (host-side cast, ~0.2% rms each), halving the two
dominant HBM streams.  H/W enter the G matmul in fp8-e4m3 with W pre-scaled
by 1024 (descaled exactly in the PSUM->SBUF copy); G only feeds the dist
term, which contributes ~0.3% of the output magnitude, so fp8's ~3% error
on G is negligible.  Measured rel err ~3.5e-3 overall.

On-core algorithm: one augmented matmul produces -beta*dist directly:
    lhsT = [-beta*G_i; -beta*|G_i|^2; -beta]   (K padded 34 -> 128, zeros)
    rhs  = [-2*G_j; 1; |G_j|^2]
    psum[i,j] = -beta*dist[i,j]
then per 128-row i-tile the PSUM is turned into clip(bt - beta*max(dist,0))
with the work spread over ACT (Relu from PSUM), DVE (bf16 subtract, STT,
clamp) and Pool (bf16 clamp), so the pipeline stays close to the DMA drain
rate (1 MiB/tile at 360 B/ns).

All load/store DMAs are issued on the sync (SP) queue in priority order --
ht x4 (chunked by j-columns so each G PSUM completes as soon as its own
chunk lands), B_prev x7, then stores interleaved with the deferred last
B_prev tile -- (wt goes via SWDGE on the Pool queue), so the head is never
starved by prefetch and early stores enter the DMA FIFO ahead of loads
they do not depend on.

SBUF partition-offset rule: sub-128-partition accesses must start at a
multiple of 32, so the two augmentation rows live at partitions 32 and 64
(rows 33..63 and 65..127 stay zero and contribute nothing to the matmul).
"""

import os
import sys

# The bass runtime drives the NeuronCores through the jax "axon" PJRT
# platform.  If a caller pinned JAX_PLATFORMS to cpu (common for running
# the pure-jax reference), undo that before jax is first imported.
if "jax" not in sys.modules:
    _jp = os.environ.get("JAX_PLATFORMS")
    if _jp is not None and "axon" not in _jp and "neuron" not in _jp:
        del os.environ["JAX_PLATFORMS"]

sys.path.insert(0, "/opt/trn_rl_repo")

import numpy as np

import concourse.bass as bass
import concourse.bacc as bacc
import concourse.mybir as mybir
from concourse.tile import TileContext
from concourse.bass_utils import run_bass_kernel_spmd

F32 = mybir.dt.float32
BF16 = mybir.dt.bfloat16
F8 = mybir.dt.float8e4
AF = mybir.ActivationFunctionType
ALU = mybir.AluOpType

B, N, D, K = 4, 2048, 1024, 32
HALF = N // 2            # rows per core
CLAMP = 10.0
N_CORES = 8
P = 128                  # partitions
JT = 512                 # moving free dim per matmul
NJ = N // JT             # 4 column chunks
NKC = D // P             # 8 contraction chunks for G
R1, R2 = 32, 64          # augmentation rows (must be multiples of 32)

# H/W dtype for the G matmul.  fp8 halves the ht DMA (1 MiB vs 2 MiB bf16);
# W is pre-scaled by WSCALE host-side so its ~1e-3 entries stay in fp8's
# normal range, and the scale is divided back out in the PSUM->SBUF copy.
# fp8 also enables DoubleRow matmuls (256-deep contraction per instruction).
HT8 = os.environ.get("KERNEL_HT8", "1") != "0"
H_DT = F8 if HT8 else BF16
WSCALE = 1024.0 if HT8 else 1.0
_nc_cache: dict = {}


def _build_nc(alpha: float, beta: float, loop_reps: int | None = None) -> "bass.Bass":
    # Bacc (not raw Bass): its finalize() runs the legalization passes that
    # split multi-sem waits (PE instructions have a single wait slot).
    nc = bacc.Bacc(None, num_devices=N_CORES)
    ht = nc.dram_tensor("ht", [D, N], H_DT, kind="ExternalInput")
    # wt is host-prepacked into the [partition, chunk*K] SBUF layout so the
    # DMA is one contiguous 256 B run per partition.
    wt = nc.dram_tensor("wt", [P, NKC * K], H_DT, kind="ExternalInput")
    bp_in = nc.dram_tensor("bprev", [HALF, N], BF16, kind="ExternalInput")
    out = nc.dram_tensor("out", [HALF, N], BF16, kind="ExternalOutput")

    with TileContext(nc) as tc:
        # Pools are shared across benchmark reps so PSUM/SBUF slot reuse
        # carries proper cross-rep dependencies.
        # PSUM budget: one pool of 3*[128,1024] = 6 banks.  The G phase
        # borrows two of these tiles (all four j-chunk PSUMs live at once, so
        # no recycle stalls in the head); dp=3 lets dist tile k+1's matmuls
        # start before tile k's PSUM consumers finish.
        with (
            tc.tile_pool(name="persist", bufs=1) as persist,
            tc.tile_pool(name="dpsum", bufs=3, space="PSUM") as dp,
            tc.tile_pool(name="qpsum", bufs=2, space="PSUM") as qp,
            tc.tile_pool(name="bpool", bufs=8) as bpool,
            tc.tile_pool(name="rpool", bufs=8) as rpool,
            tc.tile_pool(name="tpool", bufs=4) as tpool,
            tc.tile_pool(name="opool", bufs=8) as opool,
        ):
            pools = dict(
                persist=persist, dp=dp, qp=qp, bpool=bpool,
                rpool=rpool, tpool=tpool, opool=opool,
            )
            for _ in range(loop_reps or 1):
                _emit_body(nc, tc, pools, ht, wt, bp_in, out, alpha, beta)
    if not nc.is_finalized():
        nc.finalize()
    return nc


def _emit_body(nc, tc, pools, ht, wt, bp_in, out, alpha: float, beta: float):
    nb = -float(beta)
    persist, dp, qp = pools["persist"], pools["dp"], pools["qp"]
    bpool, rpool, tpool, opool = (
        pools["bpool"], pools["rpool"], pools["tpool"], pools["opool"]
    )

    # ---- DMA issue order: ht then all B_prev tiles on the sync queue, the
    # small wt via SWDGE on the Pool queue (so it neither delays ht's issue
    # nor its transfer; transfer requests hit the shared DMA engines in
    # roughly this order and the G phase is never starved by prefetch).
    wt_sb = persist.tile([P, NKC, K], H_DT, tag="wt_sb")
    nc.gpsimd.dma_start(out=wt_sb[:], in_=wt.rearrange("p (c k) -> p c k", c=NKC))

    # ht is chunked by j-COLUMNS (each DMA carries the full contraction for
    # one 512-column chunk, a 512 B contiguous run per descriptor), so the
    # j0/j1 G PSUMs -- the roots of the lhs barrier chain -- complete as
    # soon as their own chunk lands instead of waiting for all of ht.
    ht_sb = persist.tile([P, NKC, N], H_DT, tag="ht_sb")
    htr = ht.rearrange("(c p) j -> p c j", p=P)
    for jc in range(NJ):
        js = slice(jc * JT, (jc + 1) * JT)
        nc.sync.dma_start(out=ht_sb[:, :, js], in_=htr[:, :, js])

    # B_prev prefetch: the first six tiles up front; the last two are issued
    # after tile 0's first store (below) so early stores enter the DMA FIFO
    # ahead of them and the drain starts ~3 us sooner.  bt6/bt7 still land
    # around t=18-19 us, far before tiles 6/7 consume them.
    bts = []
    for it in range(HALF // P - 1):
        btile = bpool.tile([P, N], BF16, tag="bt")
        nc.sync.dma_start(out=btile[:], in_=bp_in[it * P : (it + 1) * P, :])
        bts.append(btile)

    def _late_bt():
        for it in (HALF // P - 1,):
            btile = bpool.tile([P, N], BF16, tag="btl")
            nc.sync.dma_start(out=btile[:], in_=bp_in[it * P : (it + 1) * P, :])
            bts.append(btile)

    # Warm the ACT function table at t~0 (emitted FIRST on the Pool queue so
    # its tiny memset precedes the big ones): the 1283 ns LoadActFuncSet must
    # overlap the ht DMA, not stall the first G-phase copy.
    warm = persist.tile([1, 1], BF16, tag="warm")
    nc.gpsimd.memset(warm[:], 0.0)
    nc.scalar.activation(warm[:], warm[:], AF.Relu)

    # ---- one-time constants / zero padding (Pool engine; off the DMA path)
    ones_sb = persist.tile([K, 1], BF16, tag="ones_sb")
    nc.gpsimd.memset(ones_sb[:], 1.0)
    # rhs_aug rows: 0..31 = -2*G_j | 32 = 1 | 64 = |G_j|^2 ; rest zero.
    # lhs_aug rows: 0..31 = -b*G_i | 32 = -b*|G_i|^2 | 64 = -b ; rest zero.
    rhs_aug = persist.tile([P, N], BF16, tag="rhs_aug")
    lhs_aug = persist.tile([P, HALF], BF16, tag="lhs_aug")
    gsq_in = persist.tile([K, N], BF16, tag="gsq_in")
    nc.gpsimd.memset(rhs_aug[:], 0.0)
    nc.gpsimd.memset(lhs_aug[:], 0.0)
    nc.gpsimd.memset(rhs_aug[R1 : R1 + 1, :], 1.0)
    nc.gpsimd.memset(lhs_aug[R2 : R2 + 1, :], nb)

    # ---------------- G phase (full G[b], computed locally) ----------------
    # psum = WSCALE * G^T[k, js]; the PSUM->SBUF copy descales and applies
    # the -2.  fp8 DoubleRow contracts a kc-pair (256 rows) per matmul.
    # PSUM->SBUF copies alternate ACT/DVE per chunk so neither engine
    # serializes the G tail.
    # Two borrowed dist-PSUM tiles hold all four j-chunk G PSUMs at once
    # (G in partitions 0..31, the gsq row-sum in partition 32 of the same
    # banks), so the G tail has no PSUM-recycle stalls.
    pga = dp.tile([P, N // 2], F32, tag="pd")
    pgb = dp.tile([P, N // 2], F32, tag="pd")
    pgs = [pga, pgb]
    # All 16 G matmuls are emitted before any PSUM consumer: PE runs in
    # program order, so interleaving the (ACT/DVE-gated) gsq ones-matmuls
    # here would head-of-line-block the later j-chunks' G matmuls.  With
    # j-chunked ht DMAs the matmuls are j-major in load order: each chunk's
    # full accumulation runs as soon as its columns land.
    for jc in range(NJ):
        js = slice(jc * JT, (jc + 1) * JT)
        pg = pgs[jc // 2][:, (jc % 2) * JT : (jc % 2 + 1) * JT]
        if HT8:
            for g in range(4):
                nc.tensor.matmul(
                    pg[0:K, :],
                    wt_sb[:, 2 * g : 2 * g + 2, :],
                    ht_sb[:, 2 * g : 2 * g + 2, js],
                    start=(g == 0),
                    stop=(g == 3),
                    perf_mode=mybir.MatmulPerfMode.DoubleRow,
                )
        else:
            for kc in range(NKC):
                nc.tensor.matmul(
                    pg[0:K, :],
                    wt_sb[:, kc, :],
                    ht_sb[:, kc, js],
                    start=(kc == 0),
                    stop=(kc == NKC - 1),
                )
    # Per-chunk consumer chains, hand-ordered to steer the list scheduler:
    # the j3 chain (psum -> square -> ones-mm -> R2) is the lhs barrier's
    # critical path, so its square reads PSUM directly (skipping the copy
    # hop) and is emitted before copy-j3; its ones-mm and R2 are emitted
    # before the other chunks'.  copies: ACT (even) / DVE (odd); squares:
    # j0 Pool from SBUF, j1/j2 ACT from PSUM, j3 DVE from PSUM; R2 descale:
    # j0/j2 ACT, j1/j3 DVE.  gsq scale varies per source (see r2_scale).
    def _copy(jc):
        js = slice(jc * JT, (jc + 1) * JT)
        pg = pgs[jc // 2][:, (jc % 2) * JT : (jc % 2 + 1) * JT]
        if jc % 2 == 0:
            nc.scalar.activation(
                rhs_aug[0:K, js], pg[0:K, :], AF.Copy, scale=-2.0 / WSCALE
            )
        else:
            nc.vector.tensor_scalar_mul(rhs_aug[0:K, js], pg[0:K, :], -2.0 / WSCALE)

    def _square(jc):
        js = slice(jc * JT, (jc + 1) * JT)
        pg = pgs[jc // 2][:, (jc % 2) * JT : (jc % 2 + 1) * JT]
        if jc == 0:
            nc.gpsimd.tensor_tensor(
                gsq_in[:, js], rhs_aug[0:K, js], rhs_aug[0:K, js], ALU.mult
            )
        elif jc == 1:
            nc.vector.tensor_tensor(
                gsq_in[:, js], rhs_aug[0:K, js], rhs_aug[0:K, js], ALU.mult
            )
        elif jc == 3:
            # TensorTensor may read only one PSUM operand on hardware, so
            # j3 squares the SBUF copy (-2G)^2 = 4 G^2 instead.
            nc.vector.tensor_tensor(
                gsq_in[:, js], rhs_aug[0:K, js], rhs_aug[0:K, js], ALU.mult
            )
        else:
            nc.scalar.activation(
                gsq_in[:, js], pg[0:K, :], AF.Square, scale=1.0 / WSCALE
            )

    pq0 = qp.tile([1, JT], F32, tag="pq")
    pq1 = qp.tile([1, JT], F32, tag="pq")
    pqs = [pq0[:], pq1[:], pga[K : K + 1, 0:JT], pga[K : K + 1, JT : 2 * JT]]

    def _r2(jc):
        js = slice(jc * JT, (jc + 1) * JT)
        nc.tensor.matmul(pqs[jc], ones_sb[:], gsq_in[:, js], start=True, stop=True)
        r2_scale = [0.25, 0.25, 1.0, 0.25][jc]
        if jc == 1:
            nc.vector.tensor_scalar_mul(rhs_aug[R2 : R2 + 1, js], pqs[jc], r2_scale)
        else:
            nc.scalar.activation(
                rhs_aug[R2 : R2 + 1, js], pqs[jc], AF.Copy, scale=r2_scale
            )

    # The host rotates each core's columns so its own row-half is always
    # columns [0:HALF): lhs_aug depends statically on the j0/j1 chains only,
    # so the dist phase starts while j2/j3's gsq work is still in flight.
    for jc in (0, 1):
        _copy(jc)
        _square(jc)
        _r2(jc)
    nc.vector.tensor_scalar_mul(
        lhs_aug[0:K, 0:HALF], rhs_aug[0:K, 0:HALF], float(beta) / 2.0
    )
    nc.vector.tensor_scalar_mul(
        lhs_aug[R1 : R1 + 1, 0:HALF], rhs_aug[R2 : R2 + 1, 0:HALF], nb
    )
    # (j2/j3 consumer chains are emitted inside the dist phase below, after
    # tile 0's first half, so PE doesn't head-of-line-block on their pq mms.)

    # ---------------- dist + EMA phase ----------------
    # Tile 0's first half is emitted between the j0/j1 and j2/j3 consumer
    # chains (see above); every (it, hh) piece is otherwise identical.
    tts = {}

    def _dist(it, hh):
        isl = slice(it * P, (it + 1) * P)
        bt = bts[it]
        if it not in tts:
            if alpha != 1.0:
                nc.vector.tensor_scalar_mul(bt[:], bt[:], float(alpha))
            tt = tpool.tile([P, N], BF16, tag="tt")
            tts[it] = tt
        tt = tts[it]
        if True:
            hs = slice(hh * (N // 2), (hh + 1) * (N // 2))
            pd = dp.tile([P, N // 2], F32, tag="pd")
            for jc2 in range(2):
                jl = slice(jc2 * JT, (jc2 + 1) * JT)
                jg = slice(hh * (N // 2) + jc2 * JT, hh * (N // 2) + (jc2 + 1) * JT)
                nc.tensor.matmul(
                    pd[:, jl], lhs_aug[:, isl], rhs_aug[:, jg], start=True, stop=True
                )
            # tt = bt + min(psum, 0) = bt - beta*max(dist, 0).  Pool cannot
            # read PSUM on hardware, so the 2048 PSUM columns are consumed
            # ACT:DVE at roughly 5:1 -- ACT Relu(-psum) on half 0 plus 704
            # columns of half 1 (DVE subtracts bt from each), a DVE STT
            # straight from PSUM on the remaining 320 -- and the clamps
            # split Pool (half 0) / DVE (half 1), balancing all three
            # engines under the ~1.9us/tile pipeline cadence.
            if hh == 0:
                r16 = rpool.tile([P, N // 2], BF16, tag="r16")
                nc.scalar.activation(r16[:], pd[:], AF.Relu, scale=-1.0)
                nc.vector.tensor_tensor(tt[:, hs], bt[:, hs], r16[:], ALU.subtract)
            else:
                RW = 704  # ACT-relu'd columns of half 1; DVE STTs the rest
                r16 = rpool.tile([P, N // 2], BF16, tag="r16")
                nc.scalar.activation(r16[:, 0:RW], pd[:, 0:RW], AF.Relu, scale=-1.0)
                nc.vector.tensor_tensor(
                    tt[:, N // 2 : N // 2 + RW], bt[:, N // 2 : N // 2 + RW],
                    r16[:, 0:RW], ALU.subtract,
                )
                nc.vector.scalar_tensor_tensor(
                    tt[:, N // 2 + RW : N], pd[:, RW : 2 * JT], 0.0,
                    bt[:, N // 2 + RW : N], ALU.min, ALU.add,
                )
            oth = opool.tile([P, N // 2], BF16, tag="oth")
            if hh == 0 and it < HALF // P - 1:
                nc.gpsimd.tensor_scalar(
                    oth[:], tt[:, hs], CLAMP, -CLAMP, ALU.min, ALU.max
                )
            else:
                # Last tile clamps entirely on DVE: the 1517 ns Pool clamp
                # would sit on the kernel's drain-critical chain.
                nc.vector.tensor_scalar(
                    oth[:], tt[:, hs], CLAMP, -CLAMP, ALU.min, ALU.max
                )
            nc.sync.dma_start(out=out[isl, hs], in_=oth[:])

    _dist(0, 0)
    for jc in (2, 3):
        _copy(jc)
        _square(jc)
        _r2(jc)
    _dist(0, 1)
    for it in range(1, HALF // P - 1):
        _dist(it, 0)
        _dist(it, 1)
    # Last tile: composite half first, so the kernel's drain chain ends on
    # the short relu->subtract->clamp path instead of the STT chain.
    _dist(HALF // P - 1, 1)
    _dist(HALF // P - 1, 0)


def _get_nc(alpha: float, beta: float) -> "bass.Bass":
    key = (alpha, beta)
    if key not in _nc_cache:
        _nc_cache[key] = _build_nc(alpha, beta)
    return _nc_cache[key]


def _make_in_maps(H, B_prev, W):
    h_np = mybir.dt.np(H_DT)
    # wt prepacked to the SBUF layout: wt_host[p, c*K+k] = WSCALE*W^T[c*P+p, k]
    wtt = (W.T * WSCALE).reshape(NKC, P, K)  # [c, p, k]
    wt_host = np.ascontiguousarray(wtt.transpose(1, 0, 2).reshape(P, NKC * K)).astype(
        h_np
    )
    bf_np = mybir.dt.np(BF16)
    in_maps = []
    for c in range(N_CORES):
        bidx, h = divmod(c, 2)
        htb = np.ascontiguousarray(H[bidx].T).astype(h_np)  # [1024, 2048]
        bp = B_prev[bidx, h * HALF : (h + 1) * HALF, :].astype(bf_np)
        if h == 1:
            # rotate columns so this core's own rows come first
            htb = np.concatenate([htb[:, HALF:], htb[:, :HALF]], axis=1)
            bp = np.concatenate([bp[:, HALF:], bp[:, :HALF]], axis=1)
        in_maps.append(
            {
                "ht": np.ascontiguousarray(htb),
                "wt": wt_host,
                "bprev": np.ascontiguousarray(bp),
            }
        )
    return in_maps


def _assemble(results) -> np.ndarray:
    out = np.empty((B, N, N), np.float32)
    for c in range(N_CORES):
        bidx, h = divmod(c, 2)
        r = results[c]["out"].astype(np.float32)
        if h == 1:
            r = np.concatenate([r[:, HALF:], r[:, :HALF]], axis=1)
        out[bidx, h * HALF : (h + 1) * HALF, :] = r
    return out


def _run(H, B_prev, W, alpha, beta, **rbk_kwargs):
    H = np.ascontiguousarray(np.asarray(H, dtype=np.float32))
    B_prev = np.ascontiguousarray(np.asarray(B_prev, dtype=np.float32))
    W = np.ascontiguousarray(np.asarray(W, dtype=np.float32))
    nc = _get_nc(float(alpha), float(beta))
    in_maps = _make_in_maps(H, B_prev, W)
    res = run_bass_kernel_spmd(nc, in_maps, list(range(N_CORES)), **rbk_kwargs)
    return _assemble(res.results), res


def kernel(H, B_prev, W, alpha, beta) -> np.ndarray:
    out, _ = _run(H, B_prev, W, alpha, beta)
    return out
